# revision 5
# baseline (speedup 1.0000x reference)
"""Trainium2 Bass kernel for nn_AutoregressiveLSA — v2 (quarter-split).

Reference math (complex, per batch b):
    Q  = WKQ @ E                      [2d, T]
    S  = E^H @ Q, keep i <= j         [T, T]
    out= WPV @ (E @ S) / rho_j        [d, T], cols 1..T-2 returned

v2 decomposition: split T into 4 quarters of L=512. With PT = (WPV E)^T:
    outT[j] = sum_{i<=j} S[i,j] PT[i]
            = Q[:,j]^T H_{q-1}  +  sum_{i in quarter(j), i<=j} S[i,j] PT[i]
    H_q = sum_{quarters a<=q} conj(E_a) @ PT_a        [2d, d]  (rank-accum)
which removes the inter-quarter portion of the score matrix (~40% of the
baseline's matmul cycles for phases B+C) and keeps every intermediate in
SBUF. All matmul operands are bf16 (same PE rate as f32r, half the DMA
and SBUF footprint); PSUM accumulation stays f32. Measured on HW:
rel err ~8e-3 vs the f32 reference (gate is 2e-2).

Karatsuba (3 real matmuls per complex product) everywhere:
  plain  a*b:      M1=ar·br M2=ai·bi M3=(ar+ai)(br+bi); Re=M1-M2, Im=M3-M1-M2
  conj(a)*b:       M1=ar·br M2=ai·bi M3=(ar-ai)(br+bi); Re=M1+M2, Im=M3-M1+M2
All operand sums/differences (es, ed, E^T variants, weight variants) are
prepared host-side and shipped packed, so each SBUF staging load is one
DMA instruction (a DMA holds its queue's sequencer for the whole
transfer, so instruction count on each queue matters more than bytes).
Walrus constraint: TensorTensor may read at most ONE PSUM operand — all
PSUM evacuations are a copy (routed to the Act engine via nc.any) plus
single-PSUM-operand adds/subs on DVE.

Sharding: data-parallel over batch, one NeuronCore per batch element.
"""

import numpy as np

import concourse.bass as bass
import concourse.mybir as mybir
import concourse.tile as tile
from concourse import bacc
from concourse.bass_utils import run_bass_kernel_spmd

F32 = mybir.dt.float32
BF16 = mybir.dt.bfloat16

# Problem dims (hardcoded per contract)
B = 8
D2 = 1024   # 2*dim, channel dim of E
T = 2048    # sequence length
D = 512     # output channel dim
P = 128
L = 512     # quarter length
QN = T // L         # 4 quarters
KC = D2 // P        # 8 k-tiles over channel dim
MB = D2 // P        # 8 m-tiles for Q rows
TBQ = L // P        # 4 seq blocks per quarter
TB = T // P         # 16 seq blocks


def _mm(nc, out, lhsT, rhs, start, stop):
    nc.tensor.matmul(out, lhsT, rhs, start=start, stop=stop)


def build_module():
    nc = bacc.Bacc(target_bir_lowering=False, trn_type="TRN2")

    # packed + pre-blocked inputs (partition-major per quarter so each
    # staging load is ONE contiguous DMA)
    eall_d = nc.dram_tensor("eall", [QN, P, KC, 3, L], BF16, kind="ExternalInput")
    ed_d = nc.dram_tensor("ed", [QN, P, KC, L], BF16, kind="ExternalInput")
    etall_d = nc.dram_tensor("etall", [QN - 1, P, TBQ, 3, D2], BF16,
                             kind="ExternalInput")
    wtall_d = nc.dram_tensor("wtall", [MB, P, 3, KC, P], BF16,
                             kind="ExternalInput")
    wvall_d = nc.dram_tensor("wvall", [P, KC, 3, D], BF16, kind="ExternalInput")
    mask_d = nc.dram_tensor("trimask", [P, P], BF16, kind="ExternalInput")
    rho_d = nc.dram_tensor("rho", [P, TB], F32, kind="ExternalInput")
    outall_d = nc.dram_tensor("outall", [T, 2, D], F32, kind="ExternalOutput")

    with tile.TileContext(nc) as tc:
        with tc.tile_pool(name="ps", bufs=2, space="PSUM") as ps, \
             tc.tile_pool(name="cst", bufs=1) as cst, \
             tc.tile_pool(name="hp", bufs=1) as hp, \
             tc.tile_pool(name="ep", bufs=2) as ep, \
             tc.tile_pool(name="edp", bufs=1) as edp, \
             tc.tile_pool(name="qp", bufs=2) as qp, \
             tc.tile_pool(name="qsp", bufs=1) as qsp, \
             tc.tile_pool(name="etp", bufs=1) as etp, \
             tc.tile_pool(name="ptp", bufs=1) as ptp, \
             tc.tile_pool(name="sp", bufs=1) as sp, \
             tc.tile_pool(name="wtp", bufs=2) as wtp, \
             tc.tile_pool(name="ev", bufs=2) as ev:

            _ctr = [0]

            def psum3(width):
                _ctr[0] += 1
                n = _ctr[0]
                t = [f"p{(3 * n + k) % 4}" for k in range(3)]
                return (ps.tile([P, 512], F32, tag=t[0], name=f"pa{n}")[:, :width],
                        ps.tile([P, 512], F32, tag=t[1], name=f"pb{n}")[:, :width],
                        ps.tile([P, 512], F32, tag=t[2], name=f"pc{n}")[:, :width])

            # ---- persistent small tensors (loads emitted after A1(0) so
            # the first wt tiles win the SWDGE queue at startup) ----
            rho_sb = cst.tile([P, TB], F32, tag="rho")
            mask_sb = cst.tile([P, P], BF16, tag="mask")
            wvall_t = cst.tile([P, KC, 3, D], BF16, tag="wvall")

            # cumulative H (bf16 accumulators, + Hs = Hr+Hi)
            hr_t = hp.tile([P, KC, D], BF16, tag="hr")
            hi_t = hp.tile([P, KC, D], BF16, tag="hi")
            hs_t = hp.tile([P, KC, D], BF16, tag="hs")

            def load_E(q, per_kc=False):
                eall_t = ep.tile([P, KC, 3, L], BF16, tag="eall", name=f"eall{q}")
                if per_kc:
                    # kc-major so A1's first matmuls start once kc0 lands
                    for kc in range(KC):
                        nc.sync.dma_start(eall_t[:, kc], eall_d[q, :, kc])
                else:
                    nc.sync.dma_start(eall_t[:], eall_d[q])
                return eall_t

            def emit_A1_m(qq, eall_t, Q, m):
                """One m-tile of Q(qq) = WKQ @ E(qq), karatsuba over kc."""
                qr_t, qi_t = Q
                wt_m = wtp.tile([P, 3, KC, P], BF16, tag="wt", name=f"wt{qq}_{m}")
                nc.gpsimd.dma_start(wt_m[:], wtall_d[m])
                pa, pb, pc = psum3(L)
                for kc in range(KC):
                    first, last = kc == 0, kc == KC - 1
                    _mm(nc, pa, wt_m[:, 0, kc], eall_t[:, kc, 0], first, last)
                    _mm(nc, pb, wt_m[:, 1, kc], eall_t[:, kc, 1], first, last)
                    _mm(nc, pc, wt_m[:, 2, kc], eall_t[:, kc, 2], first, last)
                nc.any.tensor_copy(out=qr_t[:, m], in_=pa)
                nc.vector.tensor_sub(qr_t[:, m], qr_t[:, m], pb)
                nc.any.tensor_copy(out=qi_t[:, m], in_=pc)
                nc.vector.tensor_sub(qi_t[:, m], qi_t[:, m], pa)
                nc.vector.tensor_sub(qi_t[:, m], qi_t[:, m], pb)

            # ---- prologue: quarter 0 inputs + A1(0) ----
            E_cur = load_E(0, per_kc=True)
            qr_cur = qp.tile([P, MB, L], BF16, tag="qr", name="qr0")
            qi_cur = qp.tile([P, MB, L], BF16, tag="qi", name="qi0")
            for m in range(MB):
                emit_A1_m(0, E_cur, (qr_cur, qi_cur), m)
                if m == 2:  # wv needed by A2(0); rho/mask by the jb loop
                    nc.gpsimd.dma_start(wvall_t[:], wvall_d[:])
                    nc.gpsimd.dma_start(rho_sb[:], rho_d[:])
                    nc.gpsimd.dma_start(mask_sb[:], mask_d[:])

            for q in range(QN):
                # ---- stage inputs for q+1 / this quarter's ET ----
                if q + 1 < QN:
                    E_nxt = load_E(q + 1)
                    qr_nxt = qp.tile([P, MB, L], BF16, tag="qr", name=f"qr{q+1}")
                    qi_nxt = qp.tile([P, MB, L], BF16, tag="qi", name=f"qi{q+1}")
                if q < QN - 1:
                    etall_t = etp.tile([P, TBQ, 3, D2], BF16, tag="etall",
                                       name=f"etall{q}")
                    nc.scalar.dma_start(etall_t[:], etall_d[q])

                eall_t = E_cur
                ed_t = edp.tile([P, KC, L], BF16, tag="ed", name=f"ed{q}")
                nc.sync.dma_start(ed_t[:], ed_d[q])
                qs_t = qsp.tile([P, MB, L], BF16, tag="qs", name=f"qs{q}")
                nc.vector.tensor_add(qs_t[:], qr_cur[:], qi_cur[:])

                # ---- A2(q): PT = E_q^T WV^T  [4 x [P, D] blocks] ----
                ptr_t = ptp.tile([P, TBQ, D], BF16, tag="ptr", name=f"ptr{q}")
                pti_t = ptp.tile([P, TBQ, D], BF16, tag="pti", name=f"pti{q}")
                pts_t = ptp.tile([P, TBQ, D], BF16, tag="pts", name=f"pts{q}")
                for tb in range(TBQ):
                    tbs = bass.ds(tb * P, P)
                    pa, pb, pc = psum3(D)
                    for kc in range(KC):
                        first, last = kc == 0, kc == KC - 1
                        _mm(nc, pa, eall_t[:, kc, 0, tbs], wvall_t[:, kc, 0], first, last)
                        _mm(nc, pb, eall_t[:, kc, 1, tbs], wvall_t[:, kc, 1], first, last)
                        _mm(nc, pc, eall_t[:, kc, 2, tbs], wvall_t[:, kc, 2], first, last)
                    nc.any.tensor_copy(out=ptr_t[:, tb], in_=pa)
                    nc.vector.tensor_sub(ptr_t[:, tb], ptr_t[:, tb], pb)
                    nc.any.tensor_copy(out=pti_t[:, tb], in_=pc)
                    nc.vector.tensor_sub(pti_t[:, tb], pti_t[:, tb], pa)
                    nc.vector.tensor_sub(pti_t[:, tb], pti_t[:, tb], pb)
                nc.vector.tensor_add(pts_t[:], ptr_t[:], pti_t[:])

                # ---- triangle-B(q): S row-strips (conj karatsuba) ----
                srs, sis, sss = [], [], []
                for ib in range(TBQ):
                    W = L - ib * P
                    ibs = bass.ds(ib * P, P)
                    cs = bass.ds(ib * P, W)
                    pa, pb, pc = psum3(W)
                    for kc in range(KC):
                        first, last = kc == 0, kc == KC - 1
                        _mm(nc, pa, eall_t[:, kc, 0, ibs], qr_cur[:, kc, cs], first, last)
                        _mm(nc, pb, eall_t[:, kc, 1, ibs], qi_cur[:, kc, cs], first, last)
                        _mm(nc, pc, ed_t[:, kc, ibs], qs_t[:, kc, cs], first, last)
                    sr = sp.tile([P, W], BF16, tag=f"sr{ib}", name=f"sr{q}_{ib}")
                    si = sp.tile([P, W], BF16, tag=f"si{ib}", name=f"si{q}_{ib}")
                    ss = sp.tile([P, W], BF16, tag=f"ss{ib}", name=f"ss{q}_{ib}")
                    nc.any.tensor_copy(out=sr[:], in_=pa)
                    nc.vector.tensor_add(sr[:], sr[:], pb)
                    nc.any.tensor_copy(out=si[:], in_=pc)
                    nc.vector.tensor_sub(si[:], si[:], pa)
                    nc.vector.tensor_add(si[:], si[:], pb)
                    dsl = bass.ds(0, P)  # diagonal block = first P cols
                    nc.vector.tensor_mul(sr[:, dsl], sr[:, dsl], mask_sb[:])
                    nc.vector.tensor_mul(si[:, dsl], si[:, dsl], mask_sb[:])
                    nc.vector.tensor_add(ss[:], sr[:], si[:])
                    srs.append(sr); sis.append(si); sss.append(ss)

                # ---- apply(q) + triangle-C(q), fused PSUM accumulation ----
                for jb in range(TBQ):
                    jbs = bass.ds(jb * P, P)
                    pa, pb, pc = psum3(D)
                    first = True
                    if q > 0:
                        for kt in range(KC):
                            _mm(nc, pa, qr_cur[:, kt, jbs], hr_t[:, kt], kt == 0, False)
                            _mm(nc, pb, qi_cur[:, kt, jbs], hi_t[:, kt], kt == 0, False)
                            _mm(nc, pc, qs_t[:, kt, jbs], hs_t[:, kt], kt == 0, False)
                        first = False
                    for ib in range(jb + 1):
                        off = bass.ds((jb - ib) * P, P)
                        st = first and ib == 0
                        last = ib == jb
                        _mm(nc, pa, srs[ib][:, off], ptr_t[:, ib], st, last)
                        _mm(nc, pb, sis[ib][:, off], pti_t[:, ib], st, last)
                        _mm(nc, pc, sss[ib][:, off], pts_t[:, ib], st, last)
                    gjb = q * TBQ + jb
                    out_t = ev.tile([P, 2, D], F32, tag="out", name=f"out{gjb}")
                    our, oui = out_t[:, 0], out_t[:, 1]
                    rb = rho_sb[:, gjb : gjb + 1].to_broadcast([P, D])
                    nc.any.tensor_copy(out=our, in_=pa)
                    nc.vector.tensor_sub(our, our, pb)
                    nc.any.tensor_copy(out=oui, in_=pc)
                    nc.vector.tensor_sub(oui, oui, pa)
                    nc.vector.tensor_sub(oui, oui, pb)
                    nc.vector.tensor_mul(our, our, rb)
                    nc.vector.tensor_mul(oui, oui, rb)
                    nc.sync.dma_start(outall_d[bass.ts(gjb, P)], out_t[:])

                # ---- H-update(q) interleaved with A1(q+1), m-tile by
                # m-tile: the H evacuation is DVE-heavy (6 TT per m-tile vs
                # ~2.5us of PE work), so alternating with A1's ~5us m-tile
                # groups keeps the PE fed while DVE drains ----
                if q < QN - 1:
                    for mt in range(MB):
                        mts = bass.ts(mt, P)
                        pa, pb, pc = psum3(D)
                        for kt in range(TBQ):
                            first, last = kt == 0, kt == TBQ - 1
                            _mm(nc, pa, etall_t[:, kt, 0, mts], ptr_t[:, kt], first, last)
                            _mm(nc, pb, etall_t[:, kt, 1, mts], pti_t[:, kt], first, last)
                            _mm(nc, pc, etall_t[:, kt, 2, mts], pts_t[:, kt], first, last)
                        if q == 0:
                            nc.any.tensor_copy(out=hr_t[:, mt], in_=pa)
                            nc.vector.tensor_add(hr_t[:, mt], hr_t[:, mt], pb)
                            nc.any.tensor_copy(out=hi_t[:, mt], in_=pc)
                            nc.vector.tensor_sub(hi_t[:, mt], hi_t[:, mt], pa)
                            nc.vector.tensor_add(hi_t[:, mt], hi_t[:, mt], pb)
                        else:
                            nc.vector.tensor_add(hr_t[:, mt], hr_t[:, mt], pa)
                            nc.vector.tensor_add(hr_t[:, mt], hr_t[:, mt], pb)
                            nc.vector.tensor_add(hi_t[:, mt], hi_t[:, mt], pc)
                            nc.vector.tensor_sub(hi_t[:, mt], hi_t[:, mt], pa)
                            nc.vector.tensor_add(hi_t[:, mt], hi_t[:, mt], pb)
                        nc.vector.tensor_add(hs_t[:, mt], hr_t[:, mt], hi_t[:, mt])
                        emit_A1_m(q + 1, E_nxt, (qr_nxt, qi_nxt), mt)
                    E_cur = E_nxt
                    qr_cur, qi_cur = qr_nxt, qi_nxt

    nc.compile()
    return nc


_NC_CACHE = None


def _get_module():
    global _NC_CACHE
    if _NC_CACHE is None:
        _NC_CACHE = build_module()
    return _NC_CACHE


def prep_shared(WKQ_re, WKQ_im, WPV_re, WPV_im):
    """Host-side weight prep, shared across cores (bf16, packed)."""
    import ml_dtypes
    bft = ml_dtypes.bfloat16

    def blk(w):  # WKQ^T blocked for per-m lhsT streaming
        wt = np.ascontiguousarray(w.T)            # [c, c']
        return wt.reshape(KC, P, MB, P).transpose(2, 1, 0, 3).astype(bft)

    wt3 = np.stack([blk(WKQ_re), blk(WKQ_im), blk(WKQ_re + WKQ_im)],
                   axis=2)  # [MB, P, 3, KC, P]
    # wv blocked [p, kc, 3, d]
    wv3 = np.stack([WPV_re.T, WPV_im.T, (WPV_re + WPV_im).T],
                   axis=1).astype(bft)  # [D2, 3, D]
    wv3 = wv3.reshape(KC, P, 3, D).transpose(1, 0, 2, 3)
    shared = {
        "wtall": np.ascontiguousarray(wt3),
        "wvall": np.ascontiguousarray(wv3),
        "trimask": np.triu(np.ones((P, P), np.float32)).astype(bft),
    }
    j = np.arange(T, dtype=np.float32)
    rho = 1.0 / np.maximum(j, 1.0)
    shared["rho"] = np.ascontiguousarray(rho.reshape(TB, P).T)  # [p, jb]
    return shared


def kernel(E_re, E_im, WKQ_re, WKQ_im, WPV_re, WPV_im):
    import ml_dtypes
    bft = ml_dtypes.bfloat16
    E_re = np.asarray(E_re, dtype=np.float32)
    E_im = np.asarray(E_im, dtype=np.float32)
    shared = prep_shared(np.asarray(WKQ_re, np.float32),
                         np.asarray(WKQ_im, np.float32),
                         np.asarray(WPV_re, np.float32),
                         np.asarray(WPV_im, np.float32))
    in_maps = []
    for b in range(B):
        m = dict(shared)
        erb = E_re[b].astype(bft)
        eib = E_im[b].astype(bft)
        esb = (E_re[b] + E_im[b]).astype(bft)
        edb = (E_re[b] - E_im[b]).astype(bft)
        # eall [q, p, kc, 3, l] from [3, kc, p, q, l]
        e3 = np.stack([erb, eib, esb], axis=0).reshape(3, KC, P, QN, L)
        m["eall"] = np.ascontiguousarray(e3.transpose(3, 2, 1, 0, 4))
        m["ed"] = np.ascontiguousarray(
            edb.reshape(KC, P, QN, L).transpose(2, 1, 0, 3))

        def tq(x):  # E^T quarters 0..2, blocked [q, tb, p, c]
            return x.T[: (QN - 1) * L].reshape(QN - 1, TBQ, P, D2)

        # etall [q, p, tb, 3, c] from stacked [q, tb, p, c] x3
        et3 = np.stack([tq(erb), tq(eib), tq(edb)], axis=3)  # [q, tb, p, 3, c]
        m["etall"] = np.ascontiguousarray(et3.transpose(0, 2, 1, 3, 4))
        in_maps.append(m)

    nc = _get_module()
    res = run_bass_kernel_spmd(nc, in_maps, core_ids=list(range(B)))

    out = np.empty((B, D, T - 2), dtype=np.complex64)
    for b in range(B):
        oa = res.results[b]["outall"]  # [T, 2, D]
        full = (oa[:, 0] + 1j * oa[:, 1].astype(np.complex64)).T  # [D, T]
        out[b] = full[:, 1 : T - 1]
    return out


# revision 6
# speedup vs baseline: 1.0297x; 1.0297x over previous
"""Trainium2 Bass kernel for nn_AutoregressiveLSA — v2 (quarter-split).

Reference math (complex, per batch b):
    Q  = WKQ @ E                      [2d, T]
    S  = E^H @ Q, keep i <= j         [T, T]
    out= WPV @ (E @ S) / rho_j        [d, T], cols 1..T-2 returned

v2 decomposition: split T into 4 quarters of L=512. With PT = (WPV E)^T:
    outT[j] = sum_{i<=j} S[i,j] PT[i]
            = Q[:,j]^T H_{q-1}  +  sum_{i in quarter(j), i<=j} S[i,j] PT[i]
    H_q = sum_{quarters a<=q} conj(E_a) @ PT_a        [2d, d]  (rank-accum)
which removes the inter-quarter portion of the score matrix (~40% of the
baseline's matmul cycles for phases B+C) and keeps every intermediate in
SBUF. All matmul operands are bf16 (same PE rate as f32r, half the DMA
and SBUF footprint); PSUM accumulation stays f32. Measured on HW:
rel err ~8e-3 vs the f32 reference (gate is 2e-2).

Karatsuba (3 real matmuls per complex product) everywhere:
  plain  a*b:      M1=ar·br M2=ai·bi M3=(ar+ai)(br+bi); Re=M1-M2, Im=M3-M1-M2
  conj(a)*b:       M1=ar·br M2=ai·bi M3=(ar-ai)(br+bi); Re=M1+M2, Im=M3-M1+M2
All operand sums/differences (es, ed, E^T variants, weight variants) are
prepared host-side and shipped packed, so each SBUF staging load is one
DMA instruction (a DMA holds its queue's sequencer for the whole
transfer, so instruction count on each queue matters more than bytes).
Walrus constraint: TensorTensor may read at most ONE PSUM operand — all
PSUM evacuations are a copy (routed to the Act engine via nc.any) plus
single-PSUM-operand adds/subs on DVE.

Sharding: data-parallel over batch, one NeuronCore per batch element.
"""

import numpy as np

import concourse.bass as bass
import concourse.mybir as mybir
import concourse.tile as tile
from concourse import bacc
from concourse.bass_utils import run_bass_kernel_spmd

F32 = mybir.dt.float32
BF16 = mybir.dt.bfloat16

# Problem dims (hardcoded per contract)
B = 8
D2 = 1024   # 2*dim, channel dim of E
T = 2048    # sequence length
D = 512     # output channel dim
P = 128
L = 512     # quarter length
QN = T // L         # 4 quarters
KC = D2 // P        # 8 k-tiles over channel dim
MB = D2 // P        # 8 m-tiles for Q rows
TBQ = L // P        # 4 seq blocks per quarter
TB = T // P         # 16 seq blocks


def _mm(nc, out, lhsT, rhs, start, stop):
    nc.tensor.matmul(out, lhsT, rhs, start=start, stop=stop)


def build_module():
    nc = bacc.Bacc(target_bir_lowering=False, trn_type="TRN2")

    # packed + pre-blocked inputs (partition-major per quarter so each
    # staging load is ONE contiguous DMA)
    eall_d = nc.dram_tensor("eall", [QN, P, KC, 3, L], BF16, kind="ExternalInput")
    ed_d = nc.dram_tensor("ed", [QN, P, KC, L], BF16, kind="ExternalInput")
    etall_d = nc.dram_tensor("etall", [QN - 1, MB, P, TBQ, 3, P], BF16,
                             kind="ExternalInput")
    wtall_d = nc.dram_tensor("wtall", [MB, P, 3, KC, P], BF16,
                             kind="ExternalInput")
    wvall_d = nc.dram_tensor("wvall", [P, KC, 3, D], BF16, kind="ExternalInput")
    mask_d = nc.dram_tensor("trimask", [P, P], BF16, kind="ExternalInput")
    rho_d = nc.dram_tensor("rho", [P, TB], F32, kind="ExternalInput")
    outall_d = nc.dram_tensor("outall", [T, 2, D], F32, kind="ExternalOutput")

    with tile.TileContext(nc) as tc:
        with tc.tile_pool(name="ps", bufs=2, space="PSUM") as ps, \
             tc.tile_pool(name="cst", bufs=1) as cst, \
             tc.tile_pool(name="hp", bufs=1) as hp, \
             tc.tile_pool(name="ep", bufs=2) as ep, \
             tc.tile_pool(name="edp", bufs=1) as edp, \
             tc.tile_pool(name="qp", bufs=2) as qp, \
             tc.tile_pool(name="qsp", bufs=1) as qsp, \
             tc.tile_pool(name="etp", bufs=2) as etp, \
             tc.tile_pool(name="ptp", bufs=1) as ptp, \
             tc.tile_pool(name="sp", bufs=1) as sp, \
             tc.tile_pool(name="wtp", bufs=4) as wtp, \
             tc.tile_pool(name="ev", bufs=2) as ev:

            _ctr = [0]

            def psum3(width):
                _ctr[0] += 1
                n = _ctr[0]
                t = [f"p{(3 * n + k) % 4}" for k in range(3)]
                return (ps.tile([P, 512], F32, tag=t[0], name=f"pa{n}")[:, :width],
                        ps.tile([P, 512], F32, tag=t[1], name=f"pb{n}")[:, :width],
                        ps.tile([P, 512], F32, tag=t[2], name=f"pc{n}")[:, :width])

            # ---- persistent small tensors (loads emitted after A1(0) so
            # the first wt tiles win the SWDGE queue at startup) ----
            rho_sb = cst.tile([P, TB], F32, tag="rho")
            mask_sb = cst.tile([P, P], BF16, tag="mask")
            wvall_t = cst.tile([P, KC, 3, D], BF16, tag="wvall")

            # cumulative H (bf16 accumulators, + Hs = Hr+Hi)
            hr_t = hp.tile([P, KC, D], BF16, tag="hr")
            hi_t = hp.tile([P, KC, D], BF16, tag="hi")
            hs_t = hp.tile([P, KC, D], BF16, tag="hs")

            def load_E(q, per_kc=False):
                eall_t = ep.tile([P, KC, 3, L], BF16, tag="eall", name=f"eall{q}")
                if per_kc:
                    # kc-major so A1's first matmuls start once kc0 lands
                    for kc in range(KC):
                        nc.sync.dma_start(eall_t[:, kc], eall_d[q, :, kc])
                else:
                    # 2-kc chunks keep each hold on the shared DMA engines
                    # short so latency-critical wt loads interleave
                    for kh in range(KC // 2):
                        nc.sync.dma_start(eall_t[:, 2 * kh : 2 * kh + 2],
                                          eall_d[q, :, 2 * kh : 2 * kh + 2])
                return eall_t

            def emit_A1_m(qq, eall_t, Q, m):
                """One m-tile of Q(qq) = WKQ @ E(qq), karatsuba over kc."""
                qr_t, qi_t = Q
                wt_m = wtp.tile([P, 3, KC, P], BF16, tag="wt", name=f"wt{qq}_{m}")
                nc.gpsimd.dma_start(wt_m[:], wtall_d[m])
                pa, pb, pc = psum3(L)
                for kc in range(KC):
                    first, last = kc == 0, kc == KC - 1
                    _mm(nc, pa, wt_m[:, 0, kc], eall_t[:, kc, 0], first, last)
                    _mm(nc, pb, wt_m[:, 1, kc], eall_t[:, kc, 1], first, last)
                    _mm(nc, pc, wt_m[:, 2, kc], eall_t[:, kc, 2], first, last)
                nc.any.tensor_copy(out=qr_t[:, m], in_=pa)
                nc.vector.tensor_sub(qr_t[:, m], qr_t[:, m], pb)
                nc.any.tensor_copy(out=qi_t[:, m], in_=pc)
                nc.vector.tensor_sub(qi_t[:, m], qi_t[:, m], pa)
                nc.vector.tensor_sub(qi_t[:, m], qi_t[:, m], pb)

            # ---- prologue: quarter 0 inputs + A1(0) ----
            E_cur = load_E(0, per_kc=True)
            qr_cur = qp.tile([P, MB, L], BF16, tag="qr", name="qr0")
            qi_cur = qp.tile([P, MB, L], BF16, tag="qi", name="qi0")
            for m in range(MB):
                emit_A1_m(0, E_cur, (qr_cur, qi_cur), m)
                if m == 2:  # wv needed by A2(0); rho/mask by the jb loop
                    nc.gpsimd.dma_start(wvall_t[:], wvall_d[:])
                    nc.gpsimd.dma_start(rho_sb[:], rho_d[:])
                    nc.gpsimd.dma_start(mask_sb[:], mask_d[:])

            for q in range(QN):
                # ---- stage inputs for q+1 / this quarter's ET ----
                if q + 1 < QN:
                    E_nxt = load_E(q + 1)
                    qr_nxt = qp.tile([P, MB, L], BF16, tag="qr", name=f"qr{q+1}")
                    qi_nxt = qp.tile([P, MB, L], BF16, tag="qi", name=f"qi{q+1}")

                eall_t = E_cur
                ed_t = edp.tile([P, KC, L], BF16, tag="ed", name=f"ed{q}")
                nc.scalar.dma_start(ed_t[:, :4], ed_d[q, :, :4])
                nc.scalar.dma_start(ed_t[:, 4:], ed_d[q, :, 4:])
                qs_t = qsp.tile([P, MB, L], BF16, tag="qs", name=f"qs{q}")
                nc.vector.tensor_add(qs_t[:], qr_cur[:], qi_cur[:])

                # ---- A2(q): PT = E_q^T WV^T  [4 x [P, D] blocks] ----
                ptr_t = ptp.tile([P, TBQ, D], BF16, tag="ptr", name=f"ptr{q}")
                pti_t = ptp.tile([P, TBQ, D], BF16, tag="pti", name=f"pti{q}")
                pts_t = ptp.tile([P, TBQ, D], BF16, tag="pts", name=f"pts{q}")
                for tb in range(TBQ):
                    tbs = bass.ds(tb * P, P)
                    pa, pb, pc = psum3(D)
                    for kc in range(KC):
                        first, last = kc == 0, kc == KC - 1
                        _mm(nc, pa, eall_t[:, kc, 0, tbs], wvall_t[:, kc, 0], first, last)
                        _mm(nc, pb, eall_t[:, kc, 1, tbs], wvall_t[:, kc, 1], first, last)
                        _mm(nc, pc, eall_t[:, kc, 2, tbs], wvall_t[:, kc, 2], first, last)
                    nc.any.tensor_copy(out=ptr_t[:, tb], in_=pa)
                    nc.vector.tensor_sub(ptr_t[:, tb], ptr_t[:, tb], pb)
                    nc.any.tensor_copy(out=pti_t[:, tb], in_=pc)
                    nc.vector.tensor_sub(pti_t[:, tb], pti_t[:, tb], pa)
                    nc.vector.tensor_sub(pti_t[:, tb], pti_t[:, tb], pb)
                nc.vector.tensor_add(pts_t[:], ptr_t[:], pti_t[:])

                # ---- triangle-B(q): S row-strips (conj karatsuba) ----
                srs, sis, sss = [], [], []
                for ib in range(TBQ):
                    W = L - ib * P
                    ibs = bass.ds(ib * P, P)
                    cs = bass.ds(ib * P, W)
                    pa, pb, pc = psum3(W)
                    for kc in range(KC):
                        first, last = kc == 0, kc == KC - 1
                        _mm(nc, pa, eall_t[:, kc, 0, ibs], qr_cur[:, kc, cs], first, last)
                        _mm(nc, pb, eall_t[:, kc, 1, ibs], qi_cur[:, kc, cs], first, last)
                        _mm(nc, pc, ed_t[:, kc, ibs], qs_t[:, kc, cs], first, last)
                    sr = sp.tile([P, W], BF16, tag=f"sr{ib}", name=f"sr{q}_{ib}")
                    si = sp.tile([P, W], BF16, tag=f"si{ib}", name=f"si{q}_{ib}")
                    ss = sp.tile([P, W], BF16, tag=f"ss{ib}", name=f"ss{q}_{ib}")
                    nc.any.tensor_copy(out=sr[:], in_=pa)
                    nc.vector.tensor_add(sr[:], sr[:], pb)
                    nc.any.tensor_copy(out=si[:], in_=pc)
                    nc.vector.tensor_sub(si[:], si[:], pa)
                    nc.vector.tensor_add(si[:], si[:], pb)
                    dsl = bass.ds(0, P)  # diagonal block = first P cols
                    nc.vector.tensor_mul(sr[:, dsl], sr[:, dsl], mask_sb[:])
                    nc.vector.tensor_mul(si[:, dsl], si[:, dsl], mask_sb[:])
                    nc.vector.tensor_add(ss[:], sr[:], si[:])
                    srs.append(sr); sis.append(si); sss.append(ss)

                # ---- apply(q) + triangle-C(q), fused PSUM accumulation ----
                for jb in range(TBQ):
                    jbs = bass.ds(jb * P, P)
                    pa, pb, pc = psum3(D)
                    first = True
                    if q > 0:
                        for kt in range(KC):
                            _mm(nc, pa, qr_cur[:, kt, jbs], hr_t[:, kt], kt == 0, False)
                            _mm(nc, pb, qi_cur[:, kt, jbs], hi_t[:, kt], kt == 0, False)
                            _mm(nc, pc, qs_t[:, kt, jbs], hs_t[:, kt], kt == 0, False)
                        first = False
                    for ib in range(jb + 1):
                        off = bass.ds((jb - ib) * P, P)
                        st = first and ib == 0
                        last = ib == jb
                        _mm(nc, pa, srs[ib][:, off], ptr_t[:, ib], st, last)
                        _mm(nc, pb, sis[ib][:, off], pti_t[:, ib], st, last)
                        _mm(nc, pc, sss[ib][:, off], pts_t[:, ib], st, last)
                    gjb = q * TBQ + jb
                    out_t = ev.tile([P, 2, D], F32, tag="out", name=f"out{gjb}")
                    our, oui = out_t[:, 0], out_t[:, 1]
                    rb = rho_sb[:, gjb : gjb + 1].to_broadcast([P, D])
                    nc.any.tensor_copy(out=our, in_=pa)
                    nc.vector.tensor_sub(our, our, pb)
                    nc.any.tensor_copy(out=oui, in_=pc)
                    nc.vector.tensor_sub(oui, oui, pa)
                    nc.vector.tensor_sub(oui, oui, pb)
                    nc.vector.tensor_mul(our, our, rb)
                    nc.vector.tensor_mul(oui, oui, rb)
                    nc.sync.dma_start(outall_d[bass.ts(gjb, P)], out_t[:])

                # ---- H-update(q) interleaved with A1(q+1), m-tile by
                # m-tile: the H evacuation is DVE-heavy (6 TT per m-tile vs
                # ~2.5us of PE work), so alternating with A1's ~5us m-tile
                # groups keeps the PE fed while DVE drains ----
                if q < QN - 1:
                    for mt in range(MB):
                        mts = bass.ts(mt, P)
                        etm_t = etp.tile([P, TBQ, 3, P], BF16, tag="etm",
                                         name=f"etm{q}_{mt}")
                        nc.scalar.dma_start(etm_t[:], etall_d[q, mt])
                        pa, pb, pc = psum3(D)
                        for kt in range(TBQ):
                            first, last = kt == 0, kt == TBQ - 1
                            _mm(nc, pa, etm_t[:, kt, 0], ptr_t[:, kt], first, last)
                            _mm(nc, pb, etm_t[:, kt, 1], pti_t[:, kt], first, last)
                            _mm(nc, pc, etm_t[:, kt, 2], pts_t[:, kt], first, last)
                        if q == 0:
                            nc.any.tensor_copy(out=hr_t[:, mt], in_=pa)
                            nc.vector.tensor_add(hr_t[:, mt], hr_t[:, mt], pb)
                            nc.any.tensor_copy(out=hi_t[:, mt], in_=pc)
                            nc.vector.tensor_sub(hi_t[:, mt], hi_t[:, mt], pa)
                            nc.vector.tensor_add(hi_t[:, mt], hi_t[:, mt], pb)
                        else:
                            nc.vector.tensor_add(hr_t[:, mt], hr_t[:, mt], pa)
                            nc.vector.tensor_add(hr_t[:, mt], hr_t[:, mt], pb)
                            nc.vector.tensor_add(hi_t[:, mt], hi_t[:, mt], pc)
                            nc.vector.tensor_sub(hi_t[:, mt], hi_t[:, mt], pa)
                            nc.vector.tensor_add(hi_t[:, mt], hi_t[:, mt], pb)
                        nc.vector.tensor_add(hs_t[:, mt], hr_t[:, mt], hi_t[:, mt])
                        emit_A1_m(q + 1, E_nxt, (qr_nxt, qi_nxt), mt)
                    E_cur = E_nxt
                    qr_cur, qi_cur = qr_nxt, qi_nxt

    nc.compile()
    return nc


_NC_CACHE = None


def _get_module():
    global _NC_CACHE
    if _NC_CACHE is None:
        _NC_CACHE = build_module()
    return _NC_CACHE


def prep_shared(WKQ_re, WKQ_im, WPV_re, WPV_im):
    """Host-side weight prep, shared across cores (bf16, packed)."""
    import ml_dtypes
    bft = ml_dtypes.bfloat16

    def blk(w):  # WKQ^T blocked for per-m lhsT streaming
        wt = np.ascontiguousarray(w.T)            # [c, c']
        return wt.reshape(KC, P, MB, P).transpose(2, 1, 0, 3).astype(bft)

    wt3 = np.stack([blk(WKQ_re), blk(WKQ_im), blk(WKQ_re + WKQ_im)],
                   axis=2)  # [MB, P, 3, KC, P]
    # wv blocked [p, kc, 3, d]
    wv3 = np.stack([WPV_re.T, WPV_im.T, (WPV_re + WPV_im).T],
                   axis=1).astype(bft)  # [D2, 3, D]
    wv3 = wv3.reshape(KC, P, 3, D).transpose(1, 0, 2, 3)
    shared = {
        "wtall": np.ascontiguousarray(wt3),
        "wvall": np.ascontiguousarray(wv3),
        "trimask": np.triu(np.ones((P, P), np.float32)).astype(bft),
    }
    j = np.arange(T, dtype=np.float32)
    rho = 1.0 / np.maximum(j, 1.0)
    shared["rho"] = np.ascontiguousarray(rho.reshape(TB, P).T)  # [p, jb]
    return shared


def kernel(E_re, E_im, WKQ_re, WKQ_im, WPV_re, WPV_im):
    import ml_dtypes
    bft = ml_dtypes.bfloat16
    E_re = np.asarray(E_re, dtype=np.float32)
    E_im = np.asarray(E_im, dtype=np.float32)
    shared = prep_shared(np.asarray(WKQ_re, np.float32),
                         np.asarray(WKQ_im, np.float32),
                         np.asarray(WPV_re, np.float32),
                         np.asarray(WPV_im, np.float32))
    in_maps = []
    for b in range(B):
        m = dict(shared)
        erb = E_re[b].astype(bft)
        eib = E_im[b].astype(bft)
        esb = (E_re[b] + E_im[b]).astype(bft)
        edb = (E_re[b] - E_im[b]).astype(bft)
        # eall [q, p, kc, 3, l] from [3, kc, p, q, l]
        e3 = np.stack([erb, eib, esb], axis=0).reshape(3, KC, P, QN, L)
        m["eall"] = np.ascontiguousarray(e3.transpose(3, 2, 1, 0, 4))
        m["ed"] = np.ascontiguousarray(
            edb.reshape(KC, P, QN, L).transpose(2, 1, 0, 3))

        def tq(x):  # E^T quarters 0..2, blocked [q, tb, p, c]
            return x.T[: (QN - 1) * L].reshape(QN - 1, TBQ, P, D2)

        # etall [q, mb, p, tb, 3, pc] from stacked [q, tb, p, 3, c] x3
        et3 = np.stack([tq(erb), tq(eib), tq(edb)], axis=3)  # [q, tb, p, 3, c]
        et3 = et3.reshape(QN - 1, TBQ, P, 3, MB, P)
        m["etall"] = np.ascontiguousarray(et3.transpose(0, 4, 2, 1, 3, 5))
        in_maps.append(m)

    nc = _get_module()
    res = run_bass_kernel_spmd(nc, in_maps, core_ids=list(range(B)))

    out = np.empty((B, D, T - 2), dtype=np.complex64)
    for b in range(B):
        oa = res.results[b]["outall"]  # [T, 2, D]
        full = (oa[:, 0] + 1j * oa[:, 1].astype(np.complex64)).T  # [D, T]
        out[b] = full[:, 1 : T - 1]
    return out


# revision 8
# speedup vs baseline: 1.0455x; 1.0154x over previous
"""Trainium2 Bass kernel for nn_AutoregressiveLSA — v2 (quarter-split).

Reference math (complex, per batch b):
    Q  = WKQ @ E                      [2d, T]
    S  = E^H @ Q, keep i <= j         [T, T]
    out= WPV @ (E @ S) / rho_j        [d, T], cols 1..T-2 returned

v2 decomposition: split T into 4 quarters of L=512. With PT = (WPV E)^T:
    outT[j] = sum_{i<=j} S[i,j] PT[i]
            = Q[:,j]^T H_{q-1}  +  sum_{i in quarter(j), i<=j} S[i,j] PT[i]
    H_q = sum_{quarters a<=q} conj(E_a) @ PT_a        [2d, d]  (rank-accum)
which removes the inter-quarter portion of the score matrix (~40% of the
baseline's matmul cycles for phases B+C) and keeps every intermediate in
SBUF. All matmul operands are bf16 (same PE rate as f32r, half the DMA
and SBUF footprint); PSUM accumulation stays f32. Measured on HW:
rel err ~8e-3 vs the f32 reference (gate is 2e-2).

Karatsuba (3 real matmuls per complex product) everywhere:
  plain  a*b:      M1=ar·br M2=ai·bi M3=(ar+ai)(br+bi); Re=M1-M2, Im=M3-M1-M2
  conj(a)*b:       M1=ar·br M2=ai·bi M3=(ar-ai)(br+bi); Re=M1+M2, Im=M3-M1+M2
All operand sums/differences (es, ed, E^T variants, weight variants) are
prepared host-side and shipped packed, so each SBUF staging load is one
DMA instruction (a DMA holds its queue's sequencer for the whole
transfer, so instruction count on each queue matters more than bytes).
Walrus constraint: TensorTensor may read at most ONE PSUM operand — all
PSUM evacuations are a copy (routed to the Act engine via nc.any) plus
single-PSUM-operand adds/subs on DVE.

Sharding: data-parallel over batch, one NeuronCore per batch element.
"""

import numpy as np

import concourse.bass as bass
import concourse.mybir as mybir
import concourse.tile as tile
from concourse import bacc
from concourse.bass_utils import run_bass_kernel_spmd

F32 = mybir.dt.float32
BF16 = mybir.dt.bfloat16

# Problem dims (hardcoded per contract)
B = 8
D2 = 1024   # 2*dim, channel dim of E
T = 2048    # sequence length
D = 512     # output channel dim
P = 128
L = 512     # quarter length
QN = T // L         # 4 quarters
KC = D2 // P        # 8 k-tiles over channel dim
MB = D2 // P        # 8 m-tiles for Q rows
TBQ = L // P        # 4 seq blocks per quarter
TB = T // P         # 16 seq blocks


def _mm(nc, out, lhsT, rhs, start, stop):
    nc.tensor.matmul(out, lhsT, rhs, start=start, stop=stop)


def build_module():
    nc = bacc.Bacc(target_bir_lowering=False, trn_type="TRN2")

    # packed + pre-blocked inputs (partition-major per quarter so each
    # staging load is ONE contiguous DMA)
    eall_d = nc.dram_tensor("eall", [QN, P, KC, 3, L], BF16, kind="ExternalInput")
    etall_d = nc.dram_tensor("etall", [QN - 1, MB, P, TBQ, 3, P], BF16,
                             kind="ExternalInput")
    wtall_d = nc.dram_tensor("wtall", [MB, P, 3, KC, P], BF16,
                             kind="ExternalInput")
    wvall_d = nc.dram_tensor("wvall", [P, KC, 3, D], BF16, kind="ExternalInput")
    mask_d = nc.dram_tensor("trimask", [P, P], BF16, kind="ExternalInput")
    rho_d = nc.dram_tensor("rho", [P, TB], F32, kind="ExternalInput")
    outall_d = nc.dram_tensor("outall", [T, 2, D], F32, kind="ExternalOutput")

    with tile.TileContext(nc) as tc:
        with tc.tile_pool(name="ps", bufs=2, space="PSUM") as ps, \
             tc.tile_pool(name="cst", bufs=1) as cst, \
             tc.tile_pool(name="hp", bufs=1) as hp, \
             tc.tile_pool(name="ep", bufs=1) as ep, \
             tc.tile_pool(name="qp", bufs=2) as qp, \
             tc.tile_pool(name="qsp", bufs=1) as qsp, \
             tc.tile_pool(name="etp", bufs=2) as etp, \
             tc.tile_pool(name="ptp", bufs=1) as ptp, \
             tc.tile_pool(name="sp", bufs=1) as sp, \
             tc.tile_pool(name="ev", bufs=2) as ev:

            _ctr = [0]

            def psum3(width):
                _ctr[0] += 1
                n = _ctr[0]
                t = [f"p{(3 * n + k) % 4}" for k in range(3)]
                return (ps.tile([P, 512], F32, tag=t[0], name=f"pa{n}")[:, :width],
                        ps.tile([P, 512], F32, tag=t[1], name=f"pb{n}")[:, :width],
                        ps.tile([P, 512], F32, tag=t[2], name=f"pc{n}")[:, :width])

            # ---- persistent small tensors (loads emitted after A1(0) so
            # the first wt tiles win the SWDGE queue at startup) ----
            rho_sb = cst.tile([P, TB], F32, tag="rho")
            mask_sb = cst.tile([P, P], BF16, tag="mask")
            wvall_t = cst.tile([P, KC, 3, D], BF16, tag="wvall")
            wt_t = cst.tile([P, MB, 3, KC, P], BF16, tag="wt")

            # cumulative H (bf16 accumulators, + Hs = Hr+Hi)
            hr_t = hp.tile([P, KC, D], BF16, tag="hr")
            hi_t = hp.tile([P, KC, D], BF16, tag="hi")
            hs_t = hp.tile([P, KC, D], BF16, tag="hs")

            def load_E(q, per_kc=False):
                eall_t = ep.tile([P, KC, 3, L], BF16, tag="eall", name=f"eall{q}")
                if per_kc:
                    # kc-major so A1's first matmuls start once kc0 lands
                    for kc in range(KC):
                        nc.sync.dma_start(eall_t[:, kc], eall_d[q, :, kc])
                else:
                    # 2-kc chunks keep each hold on the shared DMA engines
                    # short so latency-critical wt loads interleave
                    for kh in range(KC // 2):
                        nc.sync.dma_start(eall_t[:, 2 * kh : 2 * kh + 2],
                                          eall_d[q, :, 2 * kh : 2 * kh + 2])
                return eall_t

            def emit_A1_m(qq, eall_t, Q, m, load_wt=False):
                """One m-tile of Q(qq) = WKQ @ E(qq), diff-form karatsuba:
                M1=wr.er M2=wi.ei M3=(wr-wi).(er-ei); Re=M1-M2, Im=M1+M2-M3."""
                qr_t, qi_t = Q
                if load_wt:
                    nc.gpsimd.dma_start(wt_t[:, m], wtall_d[m])
                pa, pb, pc = psum3(L)
                for kc in range(KC):
                    first, last = kc == 0, kc == KC - 1
                    _mm(nc, pa, wt_t[:, m, 0, kc], eall_t[:, kc, 0], first, last)
                    _mm(nc, pb, wt_t[:, m, 1, kc], eall_t[:, kc, 1], first, last)
                    _mm(nc, pc, wt_t[:, m, 2, kc], eall_t[:, kc, 2], first, last)
                nc.any.tensor_copy(out=qr_t[:, m], in_=pa)
                nc.vector.tensor_sub(qr_t[:, m], qr_t[:, m], pb)
                nc.any.tensor_copy(out=qi_t[:, m], in_=pa)
                nc.vector.tensor_add(qi_t[:, m], qi_t[:, m], pb)
                nc.vector.tensor_sub(qi_t[:, m], qi_t[:, m], pc)

            # ---- prologue: quarter 0 inputs + A1(0) ----
            E_cur = load_E(0, per_kc=True)
            qr_cur = qp.tile([P, MB, L], BF16, tag="qr", name="qr0")
            qi_cur = qp.tile([P, MB, L], BF16, tag="qi", name="qi0")
            for m in range(MB):
                emit_A1_m(0, E_cur, (qr_cur, qi_cur), m, load_wt=True)
            for v in range(3):  # wv needed by A2(0); rho/mask by the jb loop
                nc.gpsimd.dma_start(wvall_t[:, :, v], wvall_d[:, :, v])
            nc.gpsimd.dma_start(rho_sb[:], rho_d[:])
            nc.gpsimd.dma_start(mask_sb[:], mask_d[:])

            for q in range(QN):
                eall_t = E_cur
                qs_t = qsp.tile([P, MB, L], BF16, tag="qs", name=f"qs{q}")
                nc.vector.tensor_add(qs_t[:], qr_cur[:], qi_cur[:])

                # ---- A2(q): PT = E_q^T WV^T  [4 x [P, D] blocks] ----
                ptr_t = ptp.tile([P, TBQ, D], BF16, tag="ptr", name=f"ptr{q}")
                pti_t = ptp.tile([P, TBQ, D], BF16, tag="pti", name=f"pti{q}")
                pts_t = ptp.tile([P, TBQ, D], BF16, tag="pts", name=f"pts{q}")
                for tb in range(TBQ):
                    tbs = bass.ds(tb * P, P)
                    pa, pb, pc = psum3(D)
                    for kc in range(KC):
                        first, last = kc == 0, kc == KC - 1
                        _mm(nc, pa, eall_t[:, kc, 0, tbs], wvall_t[:, kc, 0], first, last)
                        _mm(nc, pb, eall_t[:, kc, 1, tbs], wvall_t[:, kc, 1], first, last)
                        _mm(nc, pc, eall_t[:, kc, 2, tbs], wvall_t[:, kc, 2], first, last)
                    nc.any.tensor_copy(out=ptr_t[:, tb], in_=pa)
                    nc.vector.tensor_sub(ptr_t[:, tb], ptr_t[:, tb], pb)
                    nc.any.tensor_copy(out=pti_t[:, tb], in_=pa)
                    nc.vector.tensor_add(pti_t[:, tb], pti_t[:, tb], pb)
                    nc.vector.tensor_sub(pti_t[:, tb], pti_t[:, tb], pc)
                nc.vector.tensor_add(pts_t[:], ptr_t[:], pti_t[:])

                # ---- triangle-B(q): S row-strips (conj karatsuba) ----
                srs, sis, sss = [], [], []
                for ib in range(TBQ):
                    W = L - ib * P
                    ibs = bass.ds(ib * P, P)
                    cs = bass.ds(ib * P, W)
                    pa, pb, pc = psum3(W)
                    for kc in range(KC):
                        first, last = kc == 0, kc == KC - 1
                        _mm(nc, pa, eall_t[:, kc, 0, ibs], qr_cur[:, kc, cs], first, last)
                        _mm(nc, pb, eall_t[:, kc, 1, ibs], qi_cur[:, kc, cs], first, last)
                        _mm(nc, pc, eall_t[:, kc, 2, ibs], qs_t[:, kc, cs], first, last)
                    sr = sp.tile([P, W], BF16, tag=f"sr{ib}", name=f"sr{q}_{ib}")
                    si = sp.tile([P, W], BF16, tag=f"si{ib}", name=f"si{q}_{ib}")
                    ss = sp.tile([P, W], BF16, tag=f"ss{ib}", name=f"ss{q}_{ib}")
                    nc.any.tensor_copy(out=sr[:], in_=pa)
                    nc.vector.tensor_add(sr[:], sr[:], pb)
                    nc.any.tensor_copy(out=si[:], in_=pc)
                    nc.vector.tensor_sub(si[:], si[:], pa)
                    nc.vector.tensor_add(si[:], si[:], pb)
                    dsl = bass.ds(0, P)  # diagonal block = first P cols
                    nc.vector.tensor_mul(sr[:, dsl], sr[:, dsl], mask_sb[:])
                    nc.vector.tensor_mul(si[:, dsl], si[:, dsl], mask_sb[:])
                    nc.vector.tensor_add(ss[:], sr[:], si[:])
                    srs.append(sr); sis.append(si); sss.append(ss)

                # ---- stage E(q+1)/Q(q+1) for the A1 in the H-interleave ----
                if q + 1 < QN:
                    E_nxt = load_E(q + 1)
                    qr_nxt = qp.tile([P, MB, L], BF16, tag="qr", name=f"qr{q+1}")
                    qi_nxt = qp.tile([P, MB, L], BF16, tag="qi", name=f"qi{q+1}")

                # ---- apply(q) + triangle-C(q), fused PSUM accumulation ----
                for jb in range(TBQ):
                    jbs = bass.ds(jb * P, P)
                    pa, pb, pc = psum3(D)
                    first = True
                    if q > 0:
                        for kt in range(KC):
                            _mm(nc, pa, qr_cur[:, kt, jbs], hr_t[:, kt], kt == 0, False)
                            _mm(nc, pb, qi_cur[:, kt, jbs], hi_t[:, kt], kt == 0, False)
                            _mm(nc, pc, qs_t[:, kt, jbs], hs_t[:, kt], kt == 0, False)
                        first = False
                    for ib in range(jb + 1):
                        off = bass.ds((jb - ib) * P, P)
                        st = first and ib == 0
                        last = ib == jb
                        _mm(nc, pa, srs[ib][:, off], ptr_t[:, ib], st, last)
                        _mm(nc, pb, sis[ib][:, off], pti_t[:, ib], st, last)
                        _mm(nc, pc, sss[ib][:, off], pts_t[:, ib], st, last)
                    gjb = q * TBQ + jb
                    out_t = ev.tile([P, 2, D], F32, tag="out", name=f"out{gjb}")
                    our, oui = out_t[:, 0], out_t[:, 1]
                    rb = rho_sb[:, gjb : gjb + 1].to_broadcast([P, D])
                    nc.any.tensor_copy(out=our, in_=pa)
                    nc.vector.tensor_sub(our, our, pb)
                    nc.any.tensor_copy(out=oui, in_=pc)
                    nc.vector.tensor_sub(oui, oui, pa)
                    nc.vector.tensor_sub(oui, oui, pb)
                    nc.vector.tensor_mul(our, our, rb)
                    nc.vector.tensor_mul(oui, oui, rb)
                    nc.sync.dma_start(outall_d[bass.ts(gjb, P)], out_t[:])

                # ---- H-update(q) interleaved with A1(q+1), m-tile by
                # m-tile: the H evacuation is DVE-heavy (6 TT per m-tile vs
                # ~2.5us of PE work), so alternating with A1's ~5us m-tile
                # groups keeps the PE fed while DVE drains ----
                if q < QN - 1:
                    for mt in range(MB):
                        mts = bass.ts(mt, P)
                        etm_t = etp.tile([P, TBQ, 3, P], BF16, tag="etm",
                                         name=f"etm{q}_{mt}")
                        nc.scalar.dma_start(etm_t[:], etall_d[q, mt])
                        pa, pb, pc = psum3(D)
                        for kt in range(TBQ):
                            first, last = kt == 0, kt == TBQ - 1
                            _mm(nc, pa, etm_t[:, kt, 0], ptr_t[:, kt], first, last)
                            _mm(nc, pb, etm_t[:, kt, 1], pti_t[:, kt], first, last)
                            _mm(nc, pc, etm_t[:, kt, 2], pts_t[:, kt], first, last)
                        if q == 0:
                            nc.any.tensor_copy(out=hr_t[:, mt], in_=pa)
                            nc.vector.tensor_add(hr_t[:, mt], hr_t[:, mt], pb)
                            nc.any.tensor_copy(out=hi_t[:, mt], in_=pc)
                            nc.vector.tensor_sub(hi_t[:, mt], hi_t[:, mt], pa)
                            nc.vector.tensor_add(hi_t[:, mt], hi_t[:, mt], pb)
                        else:
                            nc.vector.tensor_add(hr_t[:, mt], hr_t[:, mt], pa)
                            nc.vector.tensor_add(hr_t[:, mt], hr_t[:, mt], pb)
                            nc.vector.tensor_add(hi_t[:, mt], hi_t[:, mt], pc)
                            nc.vector.tensor_sub(hi_t[:, mt], hi_t[:, mt], pa)
                            nc.vector.tensor_add(hi_t[:, mt], hi_t[:, mt], pb)
                        nc.vector.tensor_add(hs_t[:, mt], hr_t[:, mt], hi_t[:, mt])
                        emit_A1_m(q + 1, E_nxt, (qr_nxt, qi_nxt), mt)
                    E_cur = E_nxt
                    qr_cur, qi_cur = qr_nxt, qi_nxt

    nc.compile()
    return nc


_NC_CACHE = None


def _get_module():
    global _NC_CACHE
    if _NC_CACHE is None:
        _NC_CACHE = build_module()
    return _NC_CACHE


def prep_shared(WKQ_re, WKQ_im, WPV_re, WPV_im):
    """Host-side weight prep, shared across cores (bf16, packed)."""
    import ml_dtypes
    bft = ml_dtypes.bfloat16

    def blk(w):  # WKQ^T blocked for per-m lhsT streaming
        wt = np.ascontiguousarray(w.T)            # [c, c']
        return wt.reshape(KC, P, MB, P).transpose(2, 1, 0, 3).astype(bft)

    wt3 = np.stack([blk(WKQ_re), blk(WKQ_im), blk(WKQ_re - WKQ_im)],
                   axis=2)  # [MB, P, 3, KC, P]
    # wv blocked [p, kc, 3, d]
    wv3 = np.stack([WPV_re.T, WPV_im.T, (WPV_re - WPV_im).T],
                   axis=1).astype(bft)  # [D2, 3, D]
    wv3 = wv3.reshape(KC, P, 3, D).transpose(1, 0, 2, 3)
    shared = {
        "wtall": np.ascontiguousarray(wt3),
        "wvall": np.ascontiguousarray(wv3),
        "trimask": np.triu(np.ones((P, P), np.float32)).astype(bft),
    }
    j = np.arange(T, dtype=np.float32)
    rho = 1.0 / np.maximum(j, 1.0)
    shared["rho"] = np.ascontiguousarray(rho.reshape(TB, P).T)  # [p, jb]
    return shared


def kernel(E_re, E_im, WKQ_re, WKQ_im, WPV_re, WPV_im):
    import ml_dtypes
    bft = ml_dtypes.bfloat16
    E_re = np.asarray(E_re, dtype=np.float32)
    E_im = np.asarray(E_im, dtype=np.float32)
    shared = prep_shared(np.asarray(WKQ_re, np.float32),
                         np.asarray(WKQ_im, np.float32),
                         np.asarray(WPV_re, np.float32),
                         np.asarray(WPV_im, np.float32))
    in_maps = []
    for b in range(B):
        m = dict(shared)
        erb = E_re[b].astype(bft)
        eib = E_im[b].astype(bft)
        edb = (E_re[b] - E_im[b]).astype(bft)
        # eall [q, p, kc, 3, l] from [3, kc, p, q, l]
        e3 = np.stack([erb, eib, edb], axis=0).reshape(3, KC, P, QN, L)
        m["eall"] = np.ascontiguousarray(e3.transpose(3, 2, 1, 0, 4))

        def tq(x):  # E^T quarters 0..2, blocked [q, tb, p, c]
            return x.T[: (QN - 1) * L].reshape(QN - 1, TBQ, P, D2)

        # etall [q, mb, p, tb, 3, pc] from stacked [q, tb, p, 3, c] x3
        et3 = np.stack([tq(erb), tq(eib), tq(edb)], axis=3)  # [q, tb, p, 3, c]
        et3 = et3.reshape(QN - 1, TBQ, P, 3, MB, P)
        m["etall"] = np.ascontiguousarray(et3.transpose(0, 4, 2, 1, 3, 5))
        in_maps.append(m)

    nc = _get_module()
    res = run_bass_kernel_spmd(nc, in_maps, core_ids=list(range(B)))

    out = np.empty((B, D, T - 2), dtype=np.complex64)
    for b in range(B):
        oa = res.results[b]["outall"]  # [T, 2, D]
        full = (oa[:, 0] + 1j * oa[:, 1].astype(np.complex64)).T  # [D, T]
        out[b] = full[:, 1 : T - 1]
    return out


# revision 9
# speedup vs baseline: 1.0457x; 1.0002x over previous
"""Trainium2 Bass kernel for nn_AutoregressiveLSA — v2 (quarter-split).

Reference math (complex, per batch b):
    Q  = WKQ @ E                      [2d, T]
    S  = E^H @ Q, keep i <= j         [T, T]
    out= WPV @ (E @ S) / rho_j        [d, T], cols 1..T-2 returned

v2 decomposition: split T into 4 quarters of L=512. With PT = (WPV E)^T:
    outT[j] = sum_{i<=j} S[i,j] PT[i]
            = Q[:,j]^T H_{q-1}  +  sum_{i in quarter(j), i<=j} S[i,j] PT[i]
    H_q = sum_{quarters a<=q} conj(E_a) @ PT_a        [2d, d]  (rank-accum)
which removes the inter-quarter portion of the score matrix (~40% of the
baseline's matmul cycles for phases B+C) and keeps every intermediate in
SBUF. All matmul operands are bf16 (same PE rate as f32r, half the DMA
and SBUF footprint); PSUM accumulation stays f32. Measured on HW:
rel err ~8e-3 vs the f32 reference (gate is 2e-2).

Karatsuba (3 real matmuls per complex product) everywhere:
  plain  a*b:      M1=ar·br M2=ai·bi M3=(ar+ai)(br+bi); Re=M1-M2, Im=M3-M1-M2
  conj(a)*b:       M1=ar·br M2=ai·bi M3=(ar-ai)(br+bi); Re=M1+M2, Im=M3-M1+M2
All operand sums/differences (es, ed, E^T variants, weight variants) are
prepared host-side and shipped packed, so each SBUF staging load is one
DMA instruction (a DMA holds its queue's sequencer for the whole
transfer, so instruction count on each queue matters more than bytes).
Walrus constraint: TensorTensor may read at most ONE PSUM operand — all
PSUM evacuations are a copy (routed to the Act engine via nc.any) plus
single-PSUM-operand adds/subs on DVE.

Sharding: data-parallel over batch, one NeuronCore per batch element.
"""

import numpy as np

import concourse.bass as bass
import concourse.mybir as mybir
import concourse.tile as tile
from concourse import bacc
from concourse.bass_utils import run_bass_kernel_spmd

F32 = mybir.dt.float32
BF16 = mybir.dt.bfloat16

# Problem dims (hardcoded per contract)
B = 8
D2 = 1024   # 2*dim, channel dim of E
T = 2048    # sequence length
D = 512     # output channel dim
P = 128
L = 512     # quarter length
QN = T // L         # 4 quarters
KC = D2 // P        # 8 k-tiles over channel dim
MB = D2 // P        # 8 m-tiles for Q rows
TBQ = L // P        # 4 seq blocks per quarter
TB = T // P         # 16 seq blocks


def _mm(nc, out, lhsT, rhs, start, stop):
    nc.tensor.matmul(out, lhsT, rhs, start=start, stop=stop)


def build_module():
    nc = bacc.Bacc(target_bir_lowering=False, trn_type="TRN2")

    # packed + pre-blocked inputs (partition-major per quarter so each
    # staging load is ONE contiguous DMA)
    eall_d = nc.dram_tensor("eall", [QN, P, KC, 3, L], BF16, kind="ExternalInput")
    etall_d = nc.dram_tensor("etall", [QN - 1, MB, P, TBQ, 3, P], BF16,
                             kind="ExternalInput")
    wtall_d = nc.dram_tensor("wtall", [MB, P, 3, KC, P], BF16,
                             kind="ExternalInput")
    wvall_d = nc.dram_tensor("wvall", [P, KC, 3, D], BF16, kind="ExternalInput")
    mask_d = nc.dram_tensor("trimask", [P, P], BF16, kind="ExternalInput")
    rho_d = nc.dram_tensor("rho", [P, TB], F32, kind="ExternalInput")
    outall_d = nc.dram_tensor("outall", [T, 2, D], F32, kind="ExternalOutput")

    with tile.TileContext(nc) as tc:
        with tc.tile_pool(name="ps", bufs=2, space="PSUM") as ps, \
             tc.tile_pool(name="cst", bufs=1) as cst, \
             tc.tile_pool(name="hp", bufs=1) as hp, \
             tc.tile_pool(name="ep", bufs=1) as ep, \
             tc.tile_pool(name="qp", bufs=2) as qp, \
             tc.tile_pool(name="qsp", bufs=1) as qsp, \
             tc.tile_pool(name="etp", bufs=2) as etp, \
             tc.tile_pool(name="ptp", bufs=1) as ptp, \
             tc.tile_pool(name="sp", bufs=1) as sp, \
             tc.tile_pool(name="ev", bufs=2) as ev:

            _ctr = [0]

            def psum3(width):
                _ctr[0] += 1
                n = _ctr[0]
                t = [f"p{(3 * n + k) % 4}" for k in range(3)]
                return (ps.tile([P, 512], F32, tag=t[0], name=f"pa{n}")[:, :width],
                        ps.tile([P, 512], F32, tag=t[1], name=f"pb{n}")[:, :width],
                        ps.tile([P, 512], F32, tag=t[2], name=f"pc{n}")[:, :width])

            # ---- persistent small tensors (loads emitted after A1(0) so
            # the first wt tiles win the SWDGE queue at startup) ----
            rho_sb = cst.tile([P, TB], F32, tag="rho")
            mask_sb = cst.tile([P, P], BF16, tag="mask")
            wvall_t = cst.tile([P, KC, 3, D], BF16, tag="wvall")
            wt_t = cst.tile([P, MB, 3, KC, P], BF16, tag="wt")

            # cumulative H (bf16 accumulators, + Hs = Hr+Hi)
            hr_t = hp.tile([P, KC, D], BF16, tag="hr")
            hi_t = hp.tile([P, KC, D], BF16, tag="hi")
            hs_t = hp.tile([P, KC, D], BF16, tag="hs")

            def load_E(q, per_kc=False):
                eall_t = ep.tile([P, KC, 3, L], BF16, tag="eall", name=f"eall{q}")
                if per_kc:
                    # kc-major so A1's first matmuls start once kc0 lands
                    for kc in range(KC):
                        nc.sync.dma_start(eall_t[:, kc], eall_d[q, :, kc])
                else:
                    # 2-kc chunks keep each hold on the shared DMA engines
                    # short so latency-critical wt loads interleave
                    for kh in range(KC // 2):
                        nc.sync.dma_start(eall_t[:, 2 * kh : 2 * kh + 2],
                                          eall_d[q, :, 2 * kh : 2 * kh + 2])
                return eall_t

            def emit_A1_m(qq, eall_t, Q, m):
                """One m-tile of Q(qq) = WKQ @ E(qq), diff-form karatsuba:
                M1=wr.er M2=wi.ei M3=(wr-wi).(er-ei); Re=M1-M2, Im=M1+M2-M3."""
                qr_t, qi_t = Q
                pa, pb, pc = psum3(L)
                for kc in range(KC):
                    first, last = kc == 0, kc == KC - 1
                    _mm(nc, pa, wt_t[:, m, 0, kc], eall_t[:, kc, 0], first, last)
                    _mm(nc, pb, wt_t[:, m, 1, kc], eall_t[:, kc, 1], first, last)
                    _mm(nc, pc, wt_t[:, m, 2, kc], eall_t[:, kc, 2], first, last)
                nc.any.tensor_copy(out=qr_t[:, m], in_=pa)
                nc.vector.tensor_sub(qr_t[:, m], qr_t[:, m], pb)
                nc.any.tensor_copy(out=qi_t[:, m], in_=pa)
                nc.vector.tensor_add(qi_t[:, m], qi_t[:, m], pb)
                nc.vector.tensor_sub(qi_t[:, m], qi_t[:, m], pc)

            # ---- prologue: quarter 0 inputs + A1(0) ----
            E_cur = load_E(0, per_kc=True)
            nc.gpsimd.dma_start(wt_t[:, 0], wtall_d[0])
            nc.gpsimd.dma_start(wt_t[:, 1], wtall_d[1])
            qr_cur = qp.tile([P, MB, L], BF16, tag="qr", name="qr0")
            qi_cur = qp.tile([P, MB, L], BF16, tag="qi", name="qi0")
            for m in range(MB):
                if m + 2 < MB:
                    nc.gpsimd.dma_start(wt_t[:, m + 2], wtall_d[m + 2])
                emit_A1_m(0, E_cur, (qr_cur, qi_cur), m)
            for v in range(3):  # wv needed by A2(0); rho/mask by the jb loop
                nc.gpsimd.dma_start(wvall_t[:, :, v], wvall_d[:, :, v])
            nc.gpsimd.dma_start(rho_sb[:], rho_d[:])
            nc.gpsimd.dma_start(mask_sb[:], mask_d[:])

            for q in range(QN):
                eall_t = E_cur
                qs_t = qsp.tile([P, MB, L], BF16, tag="qs", name=f"qs{q}")
                nc.gpsimd.tensor_add(qs_t[:], qr_cur[:], qi_cur[:])

                # ---- A2(q): PT = E_q^T WV^T  [4 x [P, D] blocks] ----
                ptr_t = ptp.tile([P, TBQ, D], BF16, tag="ptr", name=f"ptr{q}")
                pti_t = ptp.tile([P, TBQ, D], BF16, tag="pti", name=f"pti{q}")
                pts_t = ptp.tile([P, TBQ, D], BF16, tag="pts", name=f"pts{q}")
                for tb in range(TBQ):
                    tbs = bass.ds(tb * P, P)
                    pa, pb, pc = psum3(D)
                    for kc in range(KC):
                        first, last = kc == 0, kc == KC - 1
                        _mm(nc, pa, eall_t[:, kc, 0, tbs], wvall_t[:, kc, 0], first, last)
                        _mm(nc, pb, eall_t[:, kc, 1, tbs], wvall_t[:, kc, 1], first, last)
                        _mm(nc, pc, eall_t[:, kc, 2, tbs], wvall_t[:, kc, 2], first, last)
                    nc.any.tensor_copy(out=ptr_t[:, tb], in_=pa)
                    nc.vector.tensor_sub(ptr_t[:, tb], ptr_t[:, tb], pb)
                    nc.any.tensor_copy(out=pti_t[:, tb], in_=pa)
                    nc.vector.tensor_add(pti_t[:, tb], pti_t[:, tb], pb)
                    nc.vector.tensor_sub(pti_t[:, tb], pti_t[:, tb], pc)
                nc.gpsimd.tensor_add(pts_t[:], ptr_t[:], pti_t[:])

                # ---- triangle-B(q): S row-strips (conj karatsuba) ----
                srs, sis, sss = [], [], []
                for ib in range(TBQ):
                    W = L - ib * P
                    ibs = bass.ds(ib * P, P)
                    cs = bass.ds(ib * P, W)
                    pa, pb, pc = psum3(W)
                    for kc in range(KC):
                        first, last = kc == 0, kc == KC - 1
                        _mm(nc, pa, eall_t[:, kc, 0, ibs], qr_cur[:, kc, cs], first, last)
                        _mm(nc, pb, eall_t[:, kc, 1, ibs], qi_cur[:, kc, cs], first, last)
                        _mm(nc, pc, eall_t[:, kc, 2, ibs], qs_t[:, kc, cs], first, last)
                    sr = sp.tile([P, W], BF16, tag=f"sr{ib}", name=f"sr{q}_{ib}")
                    si = sp.tile([P, W], BF16, tag=f"si{ib}", name=f"si{q}_{ib}")
                    ss = sp.tile([P, W], BF16, tag=f"ss{ib}", name=f"ss{q}_{ib}")
                    nc.any.tensor_copy(out=sr[:], in_=pa)
                    nc.vector.tensor_add(sr[:], sr[:], pb)
                    nc.any.tensor_copy(out=si[:], in_=pc)
                    nc.vector.tensor_sub(si[:], si[:], pa)
                    nc.vector.tensor_add(si[:], si[:], pb)
                    dsl = bass.ds(0, P)  # diagonal block = first P cols
                    nc.vector.tensor_mul(sr[:, dsl], sr[:, dsl], mask_sb[:])
                    nc.vector.tensor_mul(si[:, dsl], si[:, dsl], mask_sb[:])
                    nc.vector.tensor_add(ss[:], sr[:], si[:])
                    srs.append(sr); sis.append(si); sss.append(ss)

                # ---- stage E(q+1)/Q(q+1) for the A1 in the H-interleave ----
                if q + 1 < QN:
                    E_nxt = load_E(q + 1)
                    qr_nxt = qp.tile([P, MB, L], BF16, tag="qr", name=f"qr{q+1}")
                    qi_nxt = qp.tile([P, MB, L], BF16, tag="qi", name=f"qi{q+1}")

                # ---- apply(q) + triangle-C(q), fused PSUM accumulation ----
                for jb in range(TBQ):
                    jbs = bass.ds(jb * P, P)
                    pa, pb, pc = psum3(D)
                    first = True
                    if q > 0:
                        for kt in range(KC):
                            _mm(nc, pa, qr_cur[:, kt, jbs], hr_t[:, kt], kt == 0, False)
                            _mm(nc, pb, qi_cur[:, kt, jbs], hi_t[:, kt], kt == 0, False)
                            _mm(nc, pc, qs_t[:, kt, jbs], hs_t[:, kt], kt == 0, False)
                        first = False
                    for ib in range(jb + 1):
                        off = bass.ds((jb - ib) * P, P)
                        st = first and ib == 0
                        last = ib == jb
                        _mm(nc, pa, srs[ib][:, off], ptr_t[:, ib], st, last)
                        _mm(nc, pb, sis[ib][:, off], pti_t[:, ib], st, last)
                        _mm(nc, pc, sss[ib][:, off], pts_t[:, ib], st, last)
                    gjb = q * TBQ + jb
                    out_t = ev.tile([P, 2, D], F32, tag="out", name=f"out{gjb}")
                    our, oui = out_t[:, 0], out_t[:, 1]
                    rb = rho_sb[:, gjb : gjb + 1].to_broadcast([P, D])
                    nc.any.tensor_copy(out=our, in_=pa)
                    nc.vector.tensor_sub(our, our, pb)
                    nc.any.tensor_copy(out=oui, in_=pc)
                    nc.vector.tensor_sub(oui, oui, pa)
                    nc.vector.tensor_sub(oui, oui, pb)
                    nc.vector.tensor_mul(our, our, rb)
                    nc.vector.tensor_mul(oui, oui, rb)
                    if gjb == TB - 1:
                        nc.sync.dma_start(outall_d[bass.ts(gjb, P), 0], our)
                        nc.gpsimd.dma_start(outall_d[bass.ts(gjb, P), 1], oui)
                    else:
                        nc.sync.dma_start(outall_d[bass.ts(gjb, P)], out_t[:])

                # ---- H-update(q) interleaved with A1(q+1), m-tile by
                # m-tile: the H evacuation is DVE-heavy (6 TT per m-tile vs
                # ~2.5us of PE work), so alternating with A1's ~5us m-tile
                # groups keeps the PE fed while DVE drains ----
                if q < QN - 1:
                    for mt in range(MB):
                        mts = bass.ts(mt, P)
                        etm_t = etp.tile([P, TBQ, 3, P], BF16, tag="etm",
                                         name=f"etm{q}_{mt}")
                        nc.scalar.dma_start(etm_t[:], etall_d[q, mt])
                        pa, pb, pc = psum3(D)
                        for kt in range(TBQ):
                            first, last = kt == 0, kt == TBQ - 1
                            _mm(nc, pa, etm_t[:, kt, 0], ptr_t[:, kt], first, last)
                            _mm(nc, pb, etm_t[:, kt, 1], pti_t[:, kt], first, last)
                            _mm(nc, pc, etm_t[:, kt, 2], pts_t[:, kt], first, last)
                        if q == 0:
                            nc.any.tensor_copy(out=hr_t[:, mt], in_=pa)
                            nc.vector.tensor_add(hr_t[:, mt], hr_t[:, mt], pb)
                            nc.any.tensor_copy(out=hi_t[:, mt], in_=pc)
                            nc.vector.tensor_sub(hi_t[:, mt], hi_t[:, mt], pa)
                            nc.vector.tensor_add(hi_t[:, mt], hi_t[:, mt], pb)
                        else:
                            nc.vector.tensor_add(hr_t[:, mt], hr_t[:, mt], pa)
                            nc.vector.tensor_add(hr_t[:, mt], hr_t[:, mt], pb)
                            nc.vector.tensor_add(hi_t[:, mt], hi_t[:, mt], pc)
                            nc.vector.tensor_sub(hi_t[:, mt], hi_t[:, mt], pa)
                            nc.vector.tensor_add(hi_t[:, mt], hi_t[:, mt], pb)
                        nc.gpsimd.tensor_add(hs_t[:, mt], hr_t[:, mt], hi_t[:, mt])
                        emit_A1_m(q + 1, E_nxt, (qr_nxt, qi_nxt), mt)
                    E_cur = E_nxt
                    qr_cur, qi_cur = qr_nxt, qi_nxt

    nc.compile()
    return nc


_NC_CACHE = None


def _get_module():
    global _NC_CACHE
    if _NC_CACHE is None:
        _NC_CACHE = build_module()
    return _NC_CACHE


def prep_shared(WKQ_re, WKQ_im, WPV_re, WPV_im):
    """Host-side weight prep, shared across cores (bf16, packed)."""
    import ml_dtypes
    bft = ml_dtypes.bfloat16

    def blk(w):  # WKQ^T blocked for per-m lhsT streaming
        wt = np.ascontiguousarray(w.T)            # [c, c']
        return wt.reshape(KC, P, MB, P).transpose(2, 1, 0, 3).astype(bft)

    wt3 = np.stack([blk(WKQ_re), blk(WKQ_im), blk(WKQ_re - WKQ_im)],
                   axis=2)  # [MB, P, 3, KC, P]
    # wv blocked [p, kc, 3, d]
    wv3 = np.stack([WPV_re.T, WPV_im.T, (WPV_re - WPV_im).T],
                   axis=1).astype(bft)  # [D2, 3, D]
    wv3 = wv3.reshape(KC, P, 3, D).transpose(1, 0, 2, 3)
    shared = {
        "wtall": np.ascontiguousarray(wt3),
        "wvall": np.ascontiguousarray(wv3),
        "trimask": np.triu(np.ones((P, P), np.float32)).astype(bft),
    }
    j = np.arange(T, dtype=np.float32)
    rho = 1.0 / np.maximum(j, 1.0)
    shared["rho"] = np.ascontiguousarray(rho.reshape(TB, P).T)  # [p, jb]
    return shared


def kernel(E_re, E_im, WKQ_re, WKQ_im, WPV_re, WPV_im):
    import ml_dtypes
    bft = ml_dtypes.bfloat16
    E_re = np.asarray(E_re, dtype=np.float32)
    E_im = np.asarray(E_im, dtype=np.float32)
    shared = prep_shared(np.asarray(WKQ_re, np.float32),
                         np.asarray(WKQ_im, np.float32),
                         np.asarray(WPV_re, np.float32),
                         np.asarray(WPV_im, np.float32))
    in_maps = []
    for b in range(B):
        m = dict(shared)
        erb = E_re[b].astype(bft)
        eib = E_im[b].astype(bft)
        edb = (E_re[b] - E_im[b]).astype(bft)
        # eall [q, p, kc, 3, l] from [3, kc, p, q, l]
        e3 = np.stack([erb, eib, edb], axis=0).reshape(3, KC, P, QN, L)
        m["eall"] = np.ascontiguousarray(e3.transpose(3, 2, 1, 0, 4))

        def tq(x):  # E^T quarters 0..2, blocked [q, tb, p, c]
            return x.T[: (QN - 1) * L].reshape(QN - 1, TBQ, P, D2)

        # etall [q, mb, p, tb, 3, pc] from stacked [q, tb, p, 3, c] x3
        et3 = np.stack([tq(erb), tq(eib), tq(edb)], axis=3)  # [q, tb, p, 3, c]
        et3 = et3.reshape(QN - 1, TBQ, P, 3, MB, P)
        m["etall"] = np.ascontiguousarray(et3.transpose(0, 4, 2, 1, 3, 5))
        in_maps.append(m)

    nc = _get_module()
    res = run_bass_kernel_spmd(nc, in_maps, core_ids=list(range(B)))

    out = np.empty((B, D, T - 2), dtype=np.complex64)
    for b in range(B):
        oa = res.results[b]["outall"]  # [T, 2, D]
        full = (oa[:, 0] + 1j * oa[:, 1].astype(np.complex64)).T  # [D, T]
        out[b] = full[:, 1 : T - 1]
    return out


# revision 12
# speedup vs baseline: 1.0512x; 1.0053x over previous
"""Trainium2 Bass kernel for nn_AutoregressiveLSA — v2 (quarter-split).

Reference math (complex, per batch b):
    Q  = WKQ @ E                      [2d, T]
    S  = E^H @ Q, keep i <= j         [T, T]
    out= WPV @ (E @ S) / rho_j        [d, T], cols 1..T-2 returned

v2 decomposition: split T into 4 quarters of L=512. With PT = (WPV E)^T:
    outT[j] = sum_{i<=j} S[i,j] PT[i]
            = Q[:,j]^T H_{q-1}  +  sum_{i in quarter(j), i<=j} S[i,j] PT[i]
    H_q = sum_{quarters a<=q} conj(E_a) @ PT_a        [2d, d]  (rank-accum)
which removes the inter-quarter portion of the score matrix (~40% of the
baseline's matmul cycles for phases B+C) and keeps every intermediate in
SBUF. All matmul operands are bf16 (same PE rate as f32r, half the DMA
and SBUF footprint); PSUM accumulation stays f32. Measured on HW:
rel err ~8e-3 vs the f32 reference (gate is 2e-2).

Karatsuba (3 real matmuls per complex product) everywhere:
  plain  a*b:      M1=ar·br M2=ai·bi M3=(ar+ai)(br+bi); Re=M1-M2, Im=M3-M1-M2
  conj(a)*b:       M1=ar·br M2=ai·bi M3=(ar-ai)(br+bi); Re=M1+M2, Im=M3-M1+M2
All operand sums/differences (es, ed, E^T variants, weight variants) are
prepared host-side and shipped packed, so each SBUF staging load is one
DMA instruction (a DMA holds its queue's sequencer for the whole
transfer, so instruction count on each queue matters more than bytes).
Walrus constraint: TensorTensor may read at most ONE PSUM operand — all
PSUM evacuations are a copy (routed to the Act engine via nc.any) plus
single-PSUM-operand adds/subs on DVE.

Sharding: data-parallel over batch, one NeuronCore per batch element.
"""

import numpy as np

import concourse.bass as bass
import concourse.mybir as mybir
import concourse.tile as tile
from concourse import bacc
from concourse.bass_utils import run_bass_kernel_spmd

F32 = mybir.dt.float32
BF16 = mybir.dt.bfloat16

# Problem dims (hardcoded per contract)
B = 8
D2 = 1024   # 2*dim, channel dim of E
T = 2048    # sequence length
D = 512     # output channel dim
P = 128
L = 512     # quarter length
QN = T // L         # 4 quarters
KC = D2 // P        # 8 k-tiles over channel dim
MB = D2 // P        # 8 m-tiles for Q rows
TBQ = L // P        # 4 seq blocks per quarter
TB = T // P         # 16 seq blocks


def _mm(nc, out, lhsT, rhs, start, stop):
    nc.tensor.matmul(out, lhsT, rhs, start=start, stop=stop)


def build_module():
    nc = bacc.Bacc(target_bir_lowering=False, trn_type="TRN2")

    # packed + pre-blocked inputs (partition-major per quarter so each
    # staging load is ONE contiguous DMA)
    eall_d = nc.dram_tensor("eall", [QN, P, KC, 3, L], BF16, kind="ExternalInput")
    etall_d = nc.dram_tensor("etall", [QN - 1, MB, P, TBQ, 3, P], BF16,
                             kind="ExternalInput")
    wtall_d = nc.dram_tensor("wtall", [MB, P, 3, KC, P], BF16,
                             kind="ExternalInput")
    wvall_d = nc.dram_tensor("wvall", [P, KC, 3, D], BF16, kind="ExternalInput")
    mask_d = nc.dram_tensor("trimask", [P, P], BF16, kind="ExternalInput")
    rho_d = nc.dram_tensor("rho", [P, TB], F32, kind="ExternalInput")
    outall_d = nc.dram_tensor("outall", [T, 2, D], F32, kind="ExternalOutput")

    with tile.TileContext(nc) as tc:
        with tc.tile_pool(name="ps", bufs=2, space="PSUM") as ps, \
             tc.tile_pool(name="cst", bufs=1) as cst, \
             tc.tile_pool(name="hp", bufs=1) as hp, \
             tc.tile_pool(name="ep", bufs=1) as ep, \
             tc.tile_pool(name="qp", bufs=2) as qp, \
             tc.tile_pool(name="qsp", bufs=1) as qsp, \
             tc.tile_pool(name="etp", bufs=2) as etp, \
             tc.tile_pool(name="ptp", bufs=1) as ptp, \
             tc.tile_pool(name="sp", bufs=1) as sp, \
             tc.tile_pool(name="ev", bufs=2) as ev:

            _ctr = [0]

            def psum3(width):
                _ctr[0] += 1
                n = _ctr[0]
                t = [f"p{(3 * n + k) % 4}" for k in range(3)]
                return (ps.tile([P, 512], F32, tag=t[0], name=f"pa{n}")[:, :width],
                        ps.tile([P, 512], F32, tag=t[1], name=f"pb{n}")[:, :width],
                        ps.tile([P, 512], F32, tag=t[2], name=f"pc{n}")[:, :width])

            # ---- persistent small tensors (loads emitted after A1(0) so
            # the first wt tiles win the SWDGE queue at startup) ----
            rho_sb = cst.tile([P, TB], F32, tag="rho")
            mask_sb = cst.tile([P, P], BF16, tag="mask")
            wvall_t = cst.tile([P, KC, 3, D], BF16, tag="wvall")
            wt_t = cst.tile([P, MB, 3, KC, P], BF16, tag="wt")

            # cumulative H (bf16 accumulators, + Hs = Hr+Hi)
            hr_t = hp.tile([P, KC, D], BF16, tag="hr")
            hi_t = hp.tile([P, KC, D], BF16, tag="hi")
            hs_t = hp.tile([P, KC, D], BF16, tag="hs")

            def load_E(q, half_cols=False):
                eall_t = ep.tile([P, KC, 3, L], BF16, tag="eall", name=f"eall{q}")
                if half_cols:
                    # kc-major half-column chunks: A1(0) pass 1 (cols 0:256)
                    # starts once the first ~550ns chunk lands, and pass 2's
                    # halves stream in during pass 1
                    for h in range(2):
                        cols = bass.ds(h * (L // 2), L // 2)
                        for kc in range(KC):
                            nc.sync.dma_start(eall_t[:, kc, :, cols],
                                              eall_d[q, :, kc, :, cols])
                else:
                    # 2-kc chunks keep each hold on the shared DMA engines
                    # short so latency-critical wt loads interleave
                    for kh in range(KC // 2):
                        nc.sync.dma_start(eall_t[:, 2 * kh : 2 * kh + 2],
                                          eall_d[q, :, 2 * kh : 2 * kh + 2])
                return eall_t

            def emit_A1_m(qq, eall_t, Q, m, cols=None):
                """One m-tile of Q(qq) = WKQ @ E(qq), diff-form karatsuba:
                M1=wr.er M2=wi.ei M3=(wr-wi).(er-ei); Re=M1-M2, Im=M1+M2-M3."""
                qr_t, qi_t = Q
                w = L if cols is None else cols[1] - cols[0]
                csl = slice(None) if cols is None else bass.ds(cols[0], w)
                pa, pb, pc = psum3(w)
                for kc in range(KC):
                    first, last = kc == 0, kc == KC - 1
                    _mm(nc, pa, wt_t[:, m, 0, kc], eall_t[:, kc, 0, csl], first, last)
                    _mm(nc, pb, wt_t[:, m, 1, kc], eall_t[:, kc, 1, csl], first, last)
                    _mm(nc, pc, wt_t[:, m, 2, kc], eall_t[:, kc, 2, csl], first, last)
                nc.any.tensor_copy(out=qr_t[:, m, csl], in_=pa)
                nc.vector.tensor_sub(qr_t[:, m, csl], qr_t[:, m, csl], pb)
                nc.any.tensor_copy(out=qi_t[:, m, csl], in_=pa)
                nc.vector.tensor_add(qi_t[:, m, csl], qi_t[:, m, csl], pb)
                nc.vector.tensor_sub(qi_t[:, m, csl], qi_t[:, m, csl], pc)

            # ---- prologue: quarter 0 inputs + A1(0). A clean delayed
            # start beats trickle-feeding: any PE gap resets the p-state
            # ramp, so wait for wt0 + the full E(0) quarter, then run
            # gap-free (wt m-tiles stream in faster than consumption) ----
            nc.gpsimd.dma_start(wt_t[:, 0], wtall_d[0])
            E_cur = load_E(0)
            nc.gpsimd.dma_start(wt_t[:, 1], wtall_d[1])
            qr_cur = qp.tile([P, MB, L], BF16, tag="qr", name="qr0")
            qi_cur = qp.tile([P, MB, L], BF16, tag="qi", name="qi0")
            for m in range(MB):
                if m + 2 < MB:
                    nc.gpsimd.dma_start(wt_t[:, m + 2], wtall_d[m + 2])
                emit_A1_m(0, E_cur, (qr_cur, qi_cur), m)
            for v in range(3):  # wv needed by A2(0); rho/mask by the jb loop
                nc.gpsimd.dma_start(wvall_t[:, :, v], wvall_d[:, :, v])
            nc.gpsimd.dma_start(rho_sb[:], rho_d[:])
            nc.gpsimd.dma_start(mask_sb[:], mask_d[:])

            for q in range(QN):
                eall_t = E_cur
                qs_t = qsp.tile([P, MB, L], BF16, tag="qs", name=f"qs{q}")
                nc.gpsimd.tensor_add(qs_t[:], qr_cur[:], qi_cur[:])

                # ---- A2(q): PT = E_q^T WV^T  [4 x [P, D] blocks] ----
                ptr_t = ptp.tile([P, TBQ, D], BF16, tag="ptr", name=f"ptr{q}")
                pti_t = ptp.tile([P, TBQ, D], BF16, tag="pti", name=f"pti{q}")
                pts_t = ptp.tile([P, TBQ, D], BF16, tag="pts", name=f"pts{q}")
                for tb in range(TBQ):
                    tbs = bass.ds(tb * P, P)
                    pa, pb, pc = psum3(D)
                    for kc in range(KC):
                        first, last = kc == 0, kc == KC - 1
                        _mm(nc, pa, eall_t[:, kc, 0, tbs], wvall_t[:, kc, 0], first, last)
                        _mm(nc, pb, eall_t[:, kc, 1, tbs], wvall_t[:, kc, 1], first, last)
                        _mm(nc, pc, eall_t[:, kc, 2, tbs], wvall_t[:, kc, 2], first, last)
                    nc.any.tensor_copy(out=ptr_t[:, tb], in_=pa)
                    nc.vector.tensor_sub(ptr_t[:, tb], ptr_t[:, tb], pb)
                    nc.any.tensor_copy(out=pti_t[:, tb], in_=pa)
                    nc.vector.tensor_add(pti_t[:, tb], pti_t[:, tb], pb)
                    nc.vector.tensor_sub(pti_t[:, tb], pti_t[:, tb], pc)
                nc.gpsimd.tensor_add(pts_t[:], ptr_t[:], pti_t[:])

                # ---- triangle-B(q): S row-strips (conj karatsuba) ----
                srs, sis, sss = [], [], []
                for ib in range(TBQ):
                    W = L - ib * P
                    ibs = bass.ds(ib * P, P)
                    cs = bass.ds(ib * P, W)
                    pa, pb, pc = psum3(W)
                    for kc in range(KC):
                        first, last = kc == 0, kc == KC - 1
                        _mm(nc, pa, eall_t[:, kc, 0, ibs], qr_cur[:, kc, cs], first, last)
                        _mm(nc, pb, eall_t[:, kc, 1, ibs], qi_cur[:, kc, cs], first, last)
                        _mm(nc, pc, eall_t[:, kc, 2, ibs], qs_t[:, kc, cs], first, last)
                    sr = sp.tile([P, W], BF16, tag=f"sr{ib}", name=f"sr{q}_{ib}")
                    si = sp.tile([P, W], BF16, tag=f"si{ib}", name=f"si{q}_{ib}")
                    ss = sp.tile([P, W], BF16, tag=f"ss{ib}", name=f"ss{q}_{ib}")
                    nc.any.tensor_copy(out=sr[:], in_=pa)
                    nc.vector.tensor_add(sr[:], sr[:], pb)
                    nc.any.tensor_copy(out=si[:], in_=pc)
                    nc.vector.tensor_sub(si[:], si[:], pa)
                    nc.vector.tensor_add(si[:], si[:], pb)
                    dsl = bass.ds(0, P)  # diagonal block = first P cols
                    nc.vector.tensor_mul(sr[:, dsl], sr[:, dsl], mask_sb[:])
                    nc.vector.tensor_mul(si[:, dsl], si[:, dsl], mask_sb[:])
                    nc.vector.tensor_add(ss[:], sr[:], si[:])
                    srs.append(sr); sis.append(si); sss.append(ss)

                # ---- stage E(q+1)/Q(q+1) for the A1 in the H-interleave ----
                if q + 1 < QN:
                    E_nxt = load_E(q + 1)
                    qr_nxt = qp.tile([P, MB, L], BF16, tag="qr", name=f"qr{q+1}")
                    qi_nxt = qp.tile([P, MB, L], BF16, tag="qi", name=f"qi{q+1}")

                # ---- apply(q) + triangle-C(q), fused PSUM accumulation ----
                for jb in range(TBQ):
                    jbs = bass.ds(jb * P, P)
                    pa, pb, pc = psum3(D)
                    first = True
                    if q > 0:
                        for kt in range(KC):
                            _mm(nc, pa, qr_cur[:, kt, jbs], hr_t[:, kt], kt == 0, False)
                            _mm(nc, pb, qi_cur[:, kt, jbs], hi_t[:, kt], kt == 0, False)
                            _mm(nc, pc, qs_t[:, kt, jbs], hs_t[:, kt], kt == 0, False)
                        first = False
                    for ib in range(jb + 1):
                        off = bass.ds((jb - ib) * P, P)
                        st = first and ib == 0
                        last = ib == jb
                        _mm(nc, pa, srs[ib][:, off], ptr_t[:, ib], st, last)
                        _mm(nc, pb, sis[ib][:, off], pti_t[:, ib], st, last)
                        _mm(nc, pc, sss[ib][:, off], pts_t[:, ib], st, last)
                    gjb = q * TBQ + jb
                    out_t = ev.tile([P, 2, D], F32, tag="out", name=f"out{gjb}")
                    our, oui = out_t[:, 0], out_t[:, 1]
                    rsc = rho_sb[:, gjb : gjb + 1]
                    nc.any.tensor_copy(out=our, in_=pa)
                    nc.vector.tensor_sub(our, our, pb)
                    nc.scalar.activation(our, our, mybir.ActivationFunctionType.Copy,
                                         scale=rsc)
                    nc.any.tensor_copy(out=oui, in_=pc)
                    nc.vector.tensor_sub(oui, oui, pa)
                    nc.vector.tensor_sub(oui, oui, pb)
                    nc.scalar.activation(oui, oui, mybir.ActivationFunctionType.Copy,
                                         scale=rsc)
                    if gjb == TB - 1:
                        nc.sync.dma_start(outall_d[bass.ts(gjb, P), 0], our)
                        nc.gpsimd.dma_start(outall_d[bass.ts(gjb, P), 1], oui)
                    else:
                        nc.sync.dma_start(outall_d[bass.ts(gjb, P)], out_t[:])

                # ---- H-update(q) interleaved with A1(q+1), m-tile by
                # m-tile: the H evacuation is DVE-heavy (6 TT per m-tile vs
                # ~2.5us of PE work), so alternating with A1's ~5us m-tile
                # groups keeps the PE fed while DVE drains ----
                if q < QN - 1:
                    for mt in range(MB):
                        mts = bass.ts(mt, P)
                        etm_t = etp.tile([P, TBQ, 3, P], BF16, tag="etm",
                                         name=f"etm{q}_{mt}")
                        nc.scalar.dma_start(etm_t[:], etall_d[q, mt])
                        pa, pb, pc = psum3(D)
                        for kt in range(TBQ):
                            first, last = kt == 0, kt == TBQ - 1
                            _mm(nc, pa, etm_t[:, kt, 0], ptr_t[:, kt], first, last)
                            _mm(nc, pb, etm_t[:, kt, 1], pti_t[:, kt], first, last)
                            _mm(nc, pc, etm_t[:, kt, 2], pts_t[:, kt], first, last)
                        if q == 0:
                            nc.any.tensor_copy(out=hr_t[:, mt], in_=pa)
                            nc.vector.tensor_add(hr_t[:, mt], hr_t[:, mt], pb)
                            nc.any.tensor_copy(out=hi_t[:, mt], in_=pc)
                            nc.vector.tensor_sub(hi_t[:, mt], hi_t[:, mt], pa)
                            nc.vector.tensor_add(hi_t[:, mt], hi_t[:, mt], pb)
                        else:
                            nc.vector.tensor_add(hr_t[:, mt], hr_t[:, mt], pa)
                            nc.vector.tensor_add(hr_t[:, mt], hr_t[:, mt], pb)
                            nc.vector.tensor_add(hi_t[:, mt], hi_t[:, mt], pc)
                            nc.vector.tensor_sub(hi_t[:, mt], hi_t[:, mt], pa)
                            nc.vector.tensor_add(hi_t[:, mt], hi_t[:, mt], pb)
                        nc.gpsimd.tensor_add(hs_t[:, mt], hr_t[:, mt], hi_t[:, mt])
                        emit_A1_m(q + 1, E_nxt, (qr_nxt, qi_nxt), mt)
                    E_cur = E_nxt
                    qr_cur, qi_cur = qr_nxt, qi_nxt

    nc.compile()
    return nc


_NC_CACHE = None


def _get_module():
    global _NC_CACHE
    if _NC_CACHE is None:
        _NC_CACHE = build_module()
    return _NC_CACHE


def prep_shared(WKQ_re, WKQ_im, WPV_re, WPV_im):
    """Host-side weight prep, shared across cores (bf16, packed)."""
    import ml_dtypes
    bft = ml_dtypes.bfloat16

    def blk(w):  # WKQ^T blocked for per-m lhsT streaming
        wt = np.ascontiguousarray(w.T)            # [c, c']
        return wt.reshape(KC, P, MB, P).transpose(2, 1, 0, 3).astype(bft)

    wt3 = np.stack([blk(WKQ_re), blk(WKQ_im), blk(WKQ_re - WKQ_im)],
                   axis=2)  # [MB, P, 3, KC, P]
    # wv blocked [p, kc, 3, d]
    wv3 = np.stack([WPV_re.T, WPV_im.T, (WPV_re - WPV_im).T],
                   axis=1).astype(bft)  # [D2, 3, D]
    wv3 = wv3.reshape(KC, P, 3, D).transpose(1, 0, 2, 3)
    shared = {
        "wtall": np.ascontiguousarray(wt3),
        "wvall": np.ascontiguousarray(wv3),
        "trimask": np.triu(np.ones((P, P), np.float32)).astype(bft),
    }
    j = np.arange(T, dtype=np.float32)
    rho = 1.0 / np.maximum(j, 1.0)
    shared["rho"] = np.ascontiguousarray(rho.reshape(TB, P).T)  # [p, jb]
    return shared


def kernel(E_re, E_im, WKQ_re, WKQ_im, WPV_re, WPV_im):
    import ml_dtypes
    bft = ml_dtypes.bfloat16
    E_re = np.asarray(E_re, dtype=np.float32)
    E_im = np.asarray(E_im, dtype=np.float32)
    shared = prep_shared(np.asarray(WKQ_re, np.float32),
                         np.asarray(WKQ_im, np.float32),
                         np.asarray(WPV_re, np.float32),
                         np.asarray(WPV_im, np.float32))
    in_maps = []
    for b in range(B):
        m = dict(shared)
        erb = E_re[b].astype(bft)
        eib = E_im[b].astype(bft)
        edb = (E_re[b] - E_im[b]).astype(bft)
        # eall [q, p, kc, 3, l] from [3, kc, p, q, l]
        e3 = np.stack([erb, eib, edb], axis=0).reshape(3, KC, P, QN, L)
        m["eall"] = np.ascontiguousarray(e3.transpose(3, 2, 1, 0, 4))

        def tq(x):  # E^T quarters 0..2, blocked [q, tb, p, c]
            return x.T[: (QN - 1) * L].reshape(QN - 1, TBQ, P, D2)

        # etall [q, mb, p, tb, 3, pc] from stacked [q, tb, p, 3, c] x3
        et3 = np.stack([tq(erb), tq(eib), tq(edb)], axis=3)  # [q, tb, p, 3, c]
        et3 = et3.reshape(QN - 1, TBQ, P, 3, MB, P)
        m["etall"] = np.ascontiguousarray(et3.transpose(0, 4, 2, 1, 3, 5))
        in_maps.append(m)

    nc = _get_module()
    res = run_bass_kernel_spmd(nc, in_maps, core_ids=list(range(B)))

    out = np.empty((B, D, T - 2), dtype=np.complex64)
    for b in range(B):
        oa = res.results[b]["outall"]  # [T, 2, D]
        full = (oa[:, 0] + 1j * oa[:, 1].astype(np.complex64)).T  # [D, T]
        out[b] = full[:, 1 : T - 1]
    return out


# revision 13
# speedup vs baseline: 1.0695x; 1.0174x over previous
"""Trainium2 Bass kernel for nn_AutoregressiveLSA — v2 (quarter-split).

Reference math (complex, per batch b):
    Q  = WKQ @ E                      [2d, T]
    S  = E^H @ Q, keep i <= j         [T, T]
    out= WPV @ (E @ S) / rho_j        [d, T], cols 1..T-2 returned

v2 decomposition: split T into 4 quarters of L=512. With PT = (WPV E)^T:
    outT[j] = sum_{i<=j} S[i,j] PT[i]
            = Q[:,j]^T H_{q-1}  +  sum_{i in quarter(j), i<=j} S[i,j] PT[i]
    H_q = sum_{quarters a<=q} conj(E_a) @ PT_a        [2d, d]  (rank-accum)
which removes the inter-quarter portion of the score matrix (~40% of the
baseline's matmul cycles for phases B+C) and keeps every intermediate in
SBUF. All matmul operands are bf16 (same PE rate as f32r, half the DMA
and SBUF footprint); PSUM accumulation stays f32. Measured on HW:
rel err ~8e-3 vs the f32 reference (gate is 2e-2).

Karatsuba (3 real matmuls per complex product) everywhere:
  plain  a*b:      M1=ar·br M2=ai·bi M3=(ar+ai)(br+bi); Re=M1-M2, Im=M3-M1-M2
  conj(a)*b:       M1=ar·br M2=ai·bi M3=(ar-ai)(br+bi); Re=M1+M2, Im=M3-M1+M2
All operand sums/differences (es, ed, E^T variants, weight variants) are
prepared host-side and shipped packed, so each SBUF staging load is one
DMA instruction (a DMA holds its queue's sequencer for the whole
transfer, so instruction count on each queue matters more than bytes).
Walrus constraint: TensorTensor may read at most ONE PSUM operand — all
PSUM evacuations are a copy (routed to the Act engine via nc.any) plus
single-PSUM-operand adds/subs on DVE.

Sharding: data-parallel over batch, one NeuronCore per batch element.
"""

import numpy as np

import concourse.bass as bass
import concourse.mybir as mybir
import concourse.tile as tile
from concourse import bacc
from concourse.bass_utils import run_bass_kernel_spmd

F32 = mybir.dt.float32
BF16 = mybir.dt.bfloat16

# Problem dims (hardcoded per contract)
B = 8
D2 = 1024   # 2*dim, channel dim of E
T = 2048    # sequence length
D = 512     # output channel dim
P = 128
L = 512     # quarter length
QN = T // L         # 4 quarters
KC = D2 // P        # 8 k-tiles over channel dim
MB = D2 // P        # 8 m-tiles for Q rows
TBQ = L // P        # 4 seq blocks per quarter
TB = T // P         # 16 seq blocks


def _mm(nc, out, lhsT, rhs, start, stop):
    nc.tensor.matmul(out, lhsT, rhs, start=start, stop=stop)


def build_module():
    nc = bacc.Bacc(target_bir_lowering=False, trn_type="TRN2")

    # packed + pre-blocked inputs (partition-major per quarter so each
    # staging load is ONE contiguous DMA)
    eall_d = nc.dram_tensor("eall", [QN, P, KC, 3, L], BF16, kind="ExternalInput")
    etall_d = nc.dram_tensor("etall", [QN - 1, MB, P, TBQ, 3, P], BF16,
                             kind="ExternalInput")
    wtall_d = nc.dram_tensor("wtall", [MB, P, 3, KC, P], BF16,
                             kind="ExternalInput")
    wvall_d = nc.dram_tensor("wvall", [P, KC, 3, D], BF16, kind="ExternalInput")
    mask_d = nc.dram_tensor("trimask", [P, P], BF16, kind="ExternalInput")
    rho_d = nc.dram_tensor("rho", [P, TB], F32, kind="ExternalInput")
    outall_d = nc.dram_tensor("outall", [T, 2, D], F32, kind="ExternalOutput")

    with tile.TileContext(nc) as tc:
        with tc.tile_pool(name="ps", bufs=2, space="PSUM") as ps, \
             tc.tile_pool(name="cst", bufs=1) as cst, \
             tc.tile_pool(name="hp", bufs=1) as hp, \
             tc.tile_pool(name="ep", bufs=1) as ep, \
             tc.tile_pool(name="qp", bufs=2) as qp, \
             tc.tile_pool(name="qsp", bufs=1) as qsp, \
             tc.tile_pool(name="etp", bufs=2) as etp, \
             tc.tile_pool(name="ptp", bufs=1) as ptp, \
             tc.tile_pool(name="sp", bufs=1) as sp, \
             tc.tile_pool(name="ev", bufs=2) as ev:

            _ctr = [0]

            def psum3(width):
                _ctr[0] += 1
                n = _ctr[0]
                t = [f"p{(3 * n + k) % 4}" for k in range(3)]
                return (ps.tile([P, 512], F32, tag=t[0], name=f"pa{n}")[:, :width],
                        ps.tile([P, 512], F32, tag=t[1], name=f"pb{n}")[:, :width],
                        ps.tile([P, 512], F32, tag=t[2], name=f"pc{n}")[:, :width])

            # ---- persistent small tensors (loads emitted after A1(0) so
            # the first wt tiles win the SWDGE queue at startup) ----
            rho_sb = cst.tile([P, TB], F32, tag="rho")
            mask_sb = cst.tile([P, P], BF16, tag="mask")
            wvall_t = cst.tile([P, KC, 3, D], BF16, tag="wvall")
            wt_t = cst.tile([P, MB, 3, KC, P], BF16, tag="wt")

            # cumulative H (bf16 accumulators, + Hs = Hr+Hi)
            hr_t = hp.tile([P, KC, D], BF16, tag="hr")
            hi_t = hp.tile([P, KC, D], BF16, tag="hi")
            hs_t = hp.tile([P, KC, D], BF16, tag="hs")

            def load_E(q, half_cols=False):
                eall_t = ep.tile([P, KC, 3, L], BF16, tag="eall", name=f"eall{q}")
                if half_cols:
                    # kc-major half-column chunks: A1(0) pass 1 (cols 0:256)
                    # starts once the first ~550ns chunk lands, and pass 2's
                    # halves stream in during pass 1
                    for h in range(2):
                        cols = bass.ds(h * (L // 2), L // 2)
                        for kc in range(KC):
                            nc.sync.dma_start(eall_t[:, kc, :, cols],
                                              eall_d[q, :, kc, :, cols])
                else:
                    # 2-kc chunks keep each hold on the shared DMA engines
                    # short so latency-critical wt loads interleave
                    for kh in range(KC // 2):
                        nc.sync.dma_start(eall_t[:, 2 * kh : 2 * kh + 2],
                                          eall_d[q, :, 2 * kh : 2 * kh + 2])
                return eall_t

            def emit_A1_m(qq, eall_t, Q, m, cols=None):
                """One m-tile of Q(qq) = WKQ @ E(qq), diff-form karatsuba:
                M1=wr.er M2=wi.ei M3=(wr-wi).(er-ei); Re=M1-M2, Im=M1+M2-M3."""
                qr_t, qi_t = Q
                w = L if cols is None else cols[1] - cols[0]
                csl = slice(None) if cols is None else bass.ds(cols[0], w)
                pa, pb, pc = psum3(w)
                for kc in range(KC):
                    first, last = kc == 0, kc == KC - 1
                    _mm(nc, pa, wt_t[:, m, 0, kc], eall_t[:, kc, 0, csl], first, last)
                    _mm(nc, pb, wt_t[:, m, 1, kc], eall_t[:, kc, 1, csl], first, last)
                    _mm(nc, pc, wt_t[:, m, 2, kc], eall_t[:, kc, 2, csl], first, last)
                nc.any.tensor_copy(out=qr_t[:, m, csl], in_=pa)
                nc.vector.tensor_sub(qr_t[:, m, csl], qr_t[:, m, csl], pb)
                nc.any.tensor_copy(out=qi_t[:, m, csl], in_=pa)
                nc.vector.tensor_add(qi_t[:, m, csl], qi_t[:, m, csl], pb)
                nc.vector.tensor_sub(qi_t[:, m, csl], qi_t[:, m, csl], pc)

            # ---- prologue: quarter 0 inputs + A1(0). A clean delayed
            # start beats trickle-feeding: any PE gap resets the p-state
            # ramp, so wait for wt0 + the full E(0) quarter, then run
            # gap-free (wt m-tiles stream in faster than consumption) ----
            nc.gpsimd.dma_start(wt_t[:, 0], wtall_d[0])
            E_cur = load_E(0)
            nc.gpsimd.dma_start(wt_t[:, 1], wtall_d[1])
            qr_cur = qp.tile([P, MB, L], BF16, tag="qr", name="qr0")
            qi_cur = qp.tile([P, MB, L], BF16, tag="qi", name="qi0")
            for m in range(MB):
                if m + 2 < MB:
                    nc.gpsimd.dma_start(wt_t[:, m + 2], wtall_d[m + 2])
                emit_A1_m(0, E_cur, (qr_cur, qi_cur), m)
            for v in range(3):  # wv needed by A2(0); rho/mask by the jb loop
                nc.gpsimd.dma_start(wvall_t[:, :, v], wvall_d[:, :, v])
            nc.gpsimd.dma_start(rho_sb[:], rho_d[:])
            nc.gpsimd.dma_start(mask_sb[:], mask_d[:])

            for q in range(QN):
                eall_t = E_cur
                qs_t = qsp.tile([P, MB, L], BF16, tag="qs", name=f"qs{q}")
                nc.gpsimd.tensor_add(qs_t[:], qr_cur[:], qi_cur[:])

                # ---- A2(q): PT = E_q^T WV^T  [4 x [P, D] blocks] ----
                ptr_t = ptp.tile([P, TBQ, D], BF16, tag="ptr", name=f"ptr{q}")
                pti_t = ptp.tile([P, TBQ, D], BF16, tag="pti", name=f"pti{q}")
                pts_t = ptp.tile([P, TBQ, D], BF16, tag="pts", name=f"pts{q}")
                for tb in range(TBQ):
                    tbs = bass.ds(tb * P, P)
                    pa, pb, pc = psum3(D)
                    for kc in range(KC):
                        first, last = kc == 0, kc == KC - 1
                        _mm(nc, pa, eall_t[:, kc, 0, tbs], wvall_t[:, kc, 0], first, last)
                        _mm(nc, pb, eall_t[:, kc, 1, tbs], wvall_t[:, kc, 1], first, last)
                        _mm(nc, pc, eall_t[:, kc, 2, tbs], wvall_t[:, kc, 2], first, last)
                    nc.any.tensor_copy(out=ptr_t[:, tb], in_=pa)
                    nc.vector.tensor_sub(ptr_t[:, tb], ptr_t[:, tb], pb)
                    nc.any.tensor_copy(out=pti_t[:, tb], in_=pa)
                    nc.vector.tensor_add(pti_t[:, tb], pti_t[:, tb], pb)
                    nc.vector.tensor_sub(pti_t[:, tb], pti_t[:, tb], pc)
                nc.gpsimd.tensor_add(pts_t[:], ptr_t[:], pti_t[:])

                # ---- triangle-B(q): S row-strips (conj karatsuba) ----
                srs, sis, sss = [], [], []
                for ib in range(TBQ):
                    W = L - ib * P
                    ibs = bass.ds(ib * P, P)
                    cs = bass.ds(ib * P, W)
                    pa, pb, pc = psum3(W)
                    for kc in range(KC):
                        first, last = kc == 0, kc == KC - 1
                        _mm(nc, pa, eall_t[:, kc, 0, ibs], qr_cur[:, kc, cs], first, last)
                        _mm(nc, pb, eall_t[:, kc, 1, ibs], qi_cur[:, kc, cs], first, last)
                        _mm(nc, pc, eall_t[:, kc, 2, ibs], qs_t[:, kc, cs], first, last)
                    sr = sp.tile([P, W], BF16, tag=f"sr{ib}", name=f"sr{q}_{ib}")
                    si = sp.tile([P, W], BF16, tag=f"si{ib}", name=f"si{q}_{ib}")
                    ss = sp.tile([P, W], BF16, tag=f"ss{ib}", name=f"ss{q}_{ib}")
                    nc.any.tensor_copy(out=sr[:], in_=pa)
                    nc.vector.tensor_add(sr[:], sr[:], pb)
                    nc.any.tensor_copy(out=si[:], in_=pc)
                    nc.vector.tensor_sub(si[:], si[:], pa)
                    nc.vector.tensor_add(si[:], si[:], pb)
                    dsl = bass.ds(0, P)  # diagonal block = first P cols
                    nc.vector.tensor_mul(sr[:, dsl], sr[:, dsl], mask_sb[:])
                    nc.vector.tensor_mul(si[:, dsl], si[:, dsl], mask_sb[:])
                    nc.vector.tensor_add(ss[:], sr[:], si[:])
                    srs.append(sr); sis.append(si); sss.append(ss)

                # ---- stage E(q+1)/Q(q+1) for the A1 in the H-interleave ----
                if q + 1 < QN:
                    E_nxt = load_E(q + 1)
                    qr_nxt = qp.tile([P, MB, L], BF16, tag="qr", name=f"qr{q+1}")
                    qi_nxt = qp.tile([P, MB, L], BF16, tag="qi", name=f"qi{q+1}")

                def emit_jb(jb):
                    jbs = bass.ds(jb * P, P)
                    pa, pb, pc = psum3(D)
                    first = True
                    if q > 0:
                        for kt in range(KC):
                            _mm(nc, pa, qr_cur[:, kt, jbs], hr_t[:, kt], kt == 0, False)
                            _mm(nc, pb, qi_cur[:, kt, jbs], hi_t[:, kt], kt == 0, False)
                            _mm(nc, pc, qs_t[:, kt, jbs], hs_t[:, kt], kt == 0, False)
                        first = False
                    for ib in range(jb + 1):
                        off = bass.ds((jb - ib) * P, P)
                        st = first and ib == 0
                        last = ib == jb
                        _mm(nc, pa, srs[ib][:, off], ptr_t[:, ib], st, last)
                        _mm(nc, pb, sis[ib][:, off], pti_t[:, ib], st, last)
                        _mm(nc, pc, sss[ib][:, off], pts_t[:, ib], st, last)
                    gjb = q * TBQ + jb
                    out_t = ev.tile([P, 2, D], F32, tag="out", name=f"out{gjb}")
                    our, oui = out_t[:, 0], out_t[:, 1]
                    rsc = rho_sb[:, gjb : gjb + 1]
                    nc.any.tensor_copy(out=our, in_=pa)
                    nc.vector.tensor_sub(our, our, pb)
                    nc.scalar.activation(our, our, mybir.ActivationFunctionType.Copy,
                                         scale=rsc)
                    nc.any.tensor_copy(out=oui, in_=pc)
                    nc.vector.tensor_sub(oui, oui, pa)
                    nc.vector.tensor_sub(oui, oui, pb)
                    nc.scalar.activation(oui, oui, mybir.ActivationFunctionType.Copy,
                                         scale=rsc)
                    if gjb == TB - 1:
                        nc.sync.dma_start(outall_d[bass.ts(gjb, P), 0], our)
                        nc.gpsimd.dma_start(outall_d[bass.ts(gjb, P), 1], oui)
                    else:
                        nc.sync.dma_start(outall_d[bass.ts(gjb, P)], out_t[:])

                if q > 0:
                    for jb in range(TBQ):
                        emit_jb(jb)

                # ---- H-update(q) interleaved with A1(q+1), m-tile by
                # m-tile: the H evacuation is DVE-heavy (6 TT per m-tile vs
                # ~2.5us of PE work), so alternating with A1's ~5us m-tile
                # groups keeps the PE fed while DVE drains ----
                if q < QN - 1:
                    for mt in range(MB):
                        if q == 0 and mt < TBQ:
                            emit_jb(mt)
                        mts = bass.ts(mt, P)
                        etm_t = etp.tile([P, TBQ, 3, P], BF16, tag="etm",
                                         name=f"etm{q}_{mt}")
                        nc.scalar.dma_start(etm_t[:], etall_d[q, mt])
                        pa, pb, pc = psum3(D)
                        for kt in range(TBQ):
                            first, last = kt == 0, kt == TBQ - 1
                            _mm(nc, pa, etm_t[:, kt, 0], ptr_t[:, kt], first, last)
                            _mm(nc, pb, etm_t[:, kt, 1], pti_t[:, kt], first, last)
                            _mm(nc, pc, etm_t[:, kt, 2], pts_t[:, kt], first, last)
                        if q == 0:
                            nc.any.tensor_copy(out=hr_t[:, mt], in_=pa)
                            nc.vector.tensor_add(hr_t[:, mt], hr_t[:, mt], pb)
                            nc.any.tensor_copy(out=hi_t[:, mt], in_=pc)
                            nc.vector.tensor_sub(hi_t[:, mt], hi_t[:, mt], pa)
                            nc.vector.tensor_add(hi_t[:, mt], hi_t[:, mt], pb)
                        else:
                            nc.vector.tensor_add(hr_t[:, mt], hr_t[:, mt], pa)
                            nc.vector.tensor_add(hr_t[:, mt], hr_t[:, mt], pb)
                            nc.vector.tensor_add(hi_t[:, mt], hi_t[:, mt], pc)
                            nc.vector.tensor_sub(hi_t[:, mt], hi_t[:, mt], pa)
                            nc.vector.tensor_add(hi_t[:, mt], hi_t[:, mt], pb)
                        nc.gpsimd.tensor_add(hs_t[:, mt], hr_t[:, mt], hi_t[:, mt])
                        emit_A1_m(q + 1, E_nxt, (qr_nxt, qi_nxt), mt)
                    E_cur = E_nxt
                    qr_cur, qi_cur = qr_nxt, qi_nxt

    nc.compile()
    return nc


_NC_CACHE = None


def _get_module():
    global _NC_CACHE
    if _NC_CACHE is None:
        _NC_CACHE = build_module()
    return _NC_CACHE


def prep_shared(WKQ_re, WKQ_im, WPV_re, WPV_im):
    """Host-side weight prep, shared across cores (bf16, packed)."""
    import ml_dtypes
    bft = ml_dtypes.bfloat16

    def blk(w):  # WKQ^T blocked for per-m lhsT streaming
        wt = np.ascontiguousarray(w.T)            # [c, c']
        return wt.reshape(KC, P, MB, P).transpose(2, 1, 0, 3).astype(bft)

    wt3 = np.stack([blk(WKQ_re), blk(WKQ_im), blk(WKQ_re - WKQ_im)],
                   axis=2)  # [MB, P, 3, KC, P]
    # wv blocked [p, kc, 3, d]
    wv3 = np.stack([WPV_re.T, WPV_im.T, (WPV_re - WPV_im).T],
                   axis=1).astype(bft)  # [D2, 3, D]
    wv3 = wv3.reshape(KC, P, 3, D).transpose(1, 0, 2, 3)
    shared = {
        "wtall": np.ascontiguousarray(wt3),
        "wvall": np.ascontiguousarray(wv3),
        "trimask": np.triu(np.ones((P, P), np.float32)).astype(bft),
    }
    j = np.arange(T, dtype=np.float32)
    rho = 1.0 / np.maximum(j, 1.0)
    shared["rho"] = np.ascontiguousarray(rho.reshape(TB, P).T)  # [p, jb]
    return shared


def kernel(E_re, E_im, WKQ_re, WKQ_im, WPV_re, WPV_im):
    import ml_dtypes
    bft = ml_dtypes.bfloat16
    E_re = np.asarray(E_re, dtype=np.float32)
    E_im = np.asarray(E_im, dtype=np.float32)
    shared = prep_shared(np.asarray(WKQ_re, np.float32),
                         np.asarray(WKQ_im, np.float32),
                         np.asarray(WPV_re, np.float32),
                         np.asarray(WPV_im, np.float32))
    in_maps = []
    for b in range(B):
        m = dict(shared)
        erb = E_re[b].astype(bft)
        eib = E_im[b].astype(bft)
        edb = (E_re[b] - E_im[b]).astype(bft)
        # eall [q, p, kc, 3, l] from [3, kc, p, q, l]
        e3 = np.stack([erb, eib, edb], axis=0).reshape(3, KC, P, QN, L)
        m["eall"] = np.ascontiguousarray(e3.transpose(3, 2, 1, 0, 4))

        def tq(x):  # E^T quarters 0..2, blocked [q, tb, p, c]
            return x.T[: (QN - 1) * L].reshape(QN - 1, TBQ, P, D2)

        # etall [q, mb, p, tb, 3, pc] from stacked [q, tb, p, 3, c] x3
        et3 = np.stack([tq(erb), tq(eib), tq(edb)], axis=3)  # [q, tb, p, 3, c]
        et3 = et3.reshape(QN - 1, TBQ, P, 3, MB, P)
        m["etall"] = np.ascontiguousarray(et3.transpose(0, 4, 2, 1, 3, 5))
        in_maps.append(m)

    nc = _get_module()
    res = run_bass_kernel_spmd(nc, in_maps, core_ids=list(range(B)))

    out = np.empty((B, D, T - 2), dtype=np.complex64)
    for b in range(B):
        oa = res.results[b]["outall"]  # [T, 2, D]
        full = (oa[:, 0] + 1j * oa[:, 1].astype(np.complex64)).T  # [D, T]
        out[b] = full[:, 1 : T - 1]
    return out
